# revision 1
# baseline (speedup 1.0000x reference)
"""DenseCapsule routing kernel for Trainium2 (Bass/Tile), 8-core data-parallel.

Problem: x [64, 8192, 8], W [8, 160], bias [160] ->
  x_hat = (x @ W + bias).reshape(64, 8192, 10, 16)
  3 dynamic-routing iterations (softmax over out_num=10, weighted sum over
  in_num=8192, squash over the 10-axis, agreement update), return
  ||outputs||_2 over out_dim -> [64, 10].

Key algebra (x_hat is never materialized):
  s[b,j,:]  = y[b,j,:] @ Wr[:,j,:]      with y = c^T @ x_aug  (tiny [10,9])
  b_logits  = x_aug @ vhat_acc^T        vhat accumulates over iterations
  softmax over j=10: c = e * (1/Z), Z via a bf16 pair-tree
  sqrt via gpsimd pow so ACT stays on ONE function table (exp only).

Precision: x_aug, c in single bf16; vhat_acc split hi/lo bf16 and the two
halves accumulated in PSUM by two back-to-back matmuls (f32 accumulate);
m0 colsum and the s/squash/vhat path in f32. End-to-end ~7e-4 rel err.

Sharding: batch 64 -> 8 cores x 8 batches. Row space for [80, *] tensors is
(j, b) = j*8+b; xT rows are (b, d) = b*9+d.
"""

from contextlib import ExitStack

import numpy as np

import concourse.bacc as bacc
import concourse.bass as bass
import concourse.mybir as mybir
import concourse.tile as tile
import concourse.bass_utils as bass_utils

f32 = mybir.dt.float32
bf16 = mybir.dt.bfloat16
AF = mybir.ActivationFunctionType
ALU = mybir.AluOpType

_DEBUG_TAPS = False

P = 128          # SBUF partitions
NH = 64          # i-chunks per batch (8192 / 128)
NB = 8           # batches per core
D = 8            # input capsule dim
DA = 9           # augmented (+ ones column)
J = 10           # out_num
KD = 16          # out_dim
KT = NB * DA     # 72 rows (b, d)
BJ = NB * J      # 80 rows (j, b) = j*8+b
IN = 8192
N_CORES = 8


def _build_nc():
    nc = bacc.Bacc(
        "TRN2", target_bir_lowering=False, debug=False, num_devices=N_CORES
    )

    x_d = nc.dram_tensor("x", [NB, IN, D], f32, kind="ExternalInput").ap()
    w_d = nc.dram_tensor("W", [D, J * KD], f32, kind="ExternalInput").ap()
    bias_d = nc.dram_tensor("bias", [J * KD], f32, kind="ExternalInput").ap()
    out_d = nc.dram_tensor("out", [BJ, 1], f32, kind="ExternalOutput").ap()
    dbg_d = None
    dbg2_d = None
    if _DEBUG_TAPS:
        dbg_d = nc.dram_tensor(
            "dbg", [2, BJ, DA], f32, kind="ExternalOutput"
        ).ap()
        dbg2_d = nc.dram_tensor(
            "dbg2", [4, BJ, KD * KT], f32, kind="ExternalOutput"
        ).ap()

    # ---- structural constants, packed into one f32 block + bf16 identity ----
    import ml_dtypes
    identbf_np = np.eye(P, dtype=np.float32).astype(ml_dtypes.bfloat16)

    # column offsets in the packed f32 const block
    C_I80, C_B80, C_BLK, C_BLKY, C_J10, C_REP = 0, 80, 160, 240, 312, 392
    CPACK_W = C_REP + KT
    cpack_np = np.zeros((P, CPACK_W), dtype=np.float32)
    cpack_np[0:BJ, C_I80:C_I80 + BJ] = np.eye(BJ, dtype=np.float32)
    for j in range(J):
        for b in range(NB):
            for j2 in range(J):
                # cB80[(j,b), (j',b')] = 1 iff b == b'
                cpack_np[j * NB + b, C_B80 + j2 * NB + b] = 1.0
    for b in range(NB):
        for d in range(DA):
            for j in range(J):
                # cBLK[(b,d), (j,b')] = 1 iff b == b'
                cpack_np[b * DA + d, C_BLK + j * NB + b] = 1.0
    for j in range(J):
        for b in range(NB):
            # cBLKY[(j,b), (b',d)] = 1 iff b' == b
            cpack_np[j * NB + b, C_BLKY + b * DA:C_BLKY + (b + 1) * DA] = 1.0
    for j in range(J):
        for b in range(NB):
            # cJ10[j', (j,b)] = 1 iff j' == j
            cpack_np[j, C_J10 + j * NB + b] = 1.0
    for b in range(NB):
        for d in range(DA):
            # cREP[d, (b,d')] = 1 iff d' == d
            cpack_np[d, C_REP + b * DA + d] = 1.0

    identbf_d = nc.inline_tensor(identbf_np, "identbf").ap()
    cpack_d = nc.inline_tensor(cpack_np, "cpack").ap()

    with tile.TileContext(nc) as tc, ExitStack() as ctx:
        sbp = ctx.enter_context(tc.tile_pool(name="sbp", bufs=1))

        def T(shape, name, dt=f32):
            return sbp.tile(shape, dt, name=name, tag=name)

        # ----- persistent SBUF tensors -----
        x_main = T([P, NB, NH, D], "x_main")          # raw DMA staging
        x_bf = T([P, NH, NB, DA], "x_bf", bf16)       # bf16 x_aug (ones col)
        xT = T([KT, NH, P], "xT", bf16)               # x_aug^T per chunk
        cIbf = T([P, P], "cIbf", bf16)                # identity (transposes)
        cpack = T([P, CPACK_W], "cpack")              # packed f32 constants
        cI80 = cpack[0:BJ, C_I80:C_I80 + BJ]
        cB80 = cpack[0:BJ, C_B80:C_B80 + BJ]
        cBLK = cpack[0:KT, C_BLK:C_BLK + BJ]
        cBLKY = cpack[0:BJ, C_BLKY:C_BLKY + KT]
        cJ10 = cpack[0:J, C_J10:C_J10 + BJ]
        cREP = cpack[0:DA, C_REP:C_REP + KT]
        W10 = T([J, DA, KD], "W10")                   # W_aug per j
        Wr = T([BJ, DA, KD], "Wr")                    # Wr[(j,b), d, k]
        WrBIGt = T([BJ, KD, NB, DA], "WrBIGt")        # masked, k-outer (f32)
        vacc = T([BJ, DA], "vacc")
        halfs = T([BJ, KD], "halfs")                  # 0.5 (gpsimd pow -> sqrt)
        part0 = T([P, NB, D], "part0")                # m0 f32 colsum partials
        onesF = T([P, 1], "onesF")                    # f32 ones column
        tenth80 = T([1, BJ], "tenth80")               # 0.1 expander row
        y0row = T([1, KT], "y0row")                   # m0 colsum row

        e_st = [None, T([P, NH, J, NB], "e1", bf16), T([P, NH, J, NB], "e2", bf16)]
        c_st = [None, T([P, NH, J, NB], "c1", bf16), T([P, NH, J, NB], "c2", bf16)]
        u5_t = [None, T([P, NH, 5, NB], "u5_1", bf16), T([P, NH, 5, NB], "u5_2", bf16)]
        v2_t = [None, T([P, NH, 2, NB], "v2_1", bf16), T([P, NH, 2, NB], "v2_2", bf16)]
        w1_t = [None, T([P, NH, NB], "w1_1", bf16), T([P, NH, NB], "w1_2", bf16)]
        Z_t = [None, T([P, NH, NB], "Z_1"), T([P, NH, NB], "Z_2")]
        Zr_t = [None, T([P, NH, NB], "Zr_1"), T([P, NH, NB], "Zr_2")]
        Zrb_t = [None, T([P, NH, NB], "Zrb_1", bf16), T([P, NH, NB], "Zrb_2", bf16)]
        blkv_t = [T([KT, 2, BJ], "blkv0", bf16), T([KT, 2, BJ], "blkv1", bf16)]

        # ----- input x: 8 contiguous per-batch DMAs over both HW queues -----
        for b in range(NB):
            eng = nc.sync if b % 2 == 0 else nc.scalar
            eng.dma_start(
                x_main[:, b, :, :],
                x_d[b].rearrange("(p h) d -> p h d", p=P),
            )

        # ----- constants -----
        nc.sync.dma_start(cpack[:, :], cpack_d[:, :])
        nc.scalar.dma_start(cIbf[:, :], identbf_d[:, :])
        # W10[j, d, k] = W_aug[d, j*16+k]
        nc.scalar.dma_start(
            W10[:, 0:D, :],
            bass.AP(tensor=w_d.tensor, offset=0,
                    ap=[[KD, J], [J * KD, D], [1, KD]]),
        )
        nc.scalar.dma_start(
            W10[:, D, :],
            bass.AP(tensor=bias_d.tensor, offset=0, ap=[[KD, J], [1, KD]]),
        )

        nc.gpsimd.memset(halfs[:, :], 0.5)
        nc.gpsimd.memset(onesF[:, :], 1.0)
        nc.gpsimd.memset(tenth80[:, :], 1.0 / J)
        nc.gpsimd.memset(x_bf[:, :, :, D], 1.0)
        # warm the gpsimd pow library during the DMA wait (the ext-isa
        # reload otherwise lands on the m0 critical chain)
        powwarm = T([BJ, 1], "powwarm")
        nc.gpsimd.tensor_tensor(
            powwarm[:, :], halfs[:, 0:1], halfs[:, 0:1], ALU.pow
        )

        # ----- bf16 cast of x (+ones) on ACT/Pool; f32 m0 colsum on DVE -----
        for b in range(NB):
            if b % 2 == 0:
                nc.scalar.copy(x_bf[:, :, b, 0:D], x_main[:, b, :, :])
            else:
                nc.gpsimd.tensor_copy(x_bf[:, :, b, 0:D], x_main[:, b, :, :])
            nc.vector.reduce_sum(
                part0[:, b, :],
                x_main[:, b, :, :].transpose([0, 2, 1]),
                axis=mybir.AxisListType.X,
            )

        with tc.tile_pool(name="yp", bufs=2, space="PSUM") as yp:
            # Wr[(j,b), d, k] = W_aug[d, j*16+k] via selector matmul
            wr_ps = yp.tile([BJ, DA, KD], f32, tag="ypsum", name="wr_ps")
            nc.tensor.matmul(
                wr_ps[:, :, :], cJ10, W10[:, :, :], start=True, stop=True
            )
            nc.vector.tensor_copy(Wr[:, :, :], wr_ps[:, :, :])

            def s_and_squash(m, y_ps):
                """s, squash -> o; returns o_sb. f32 throughout.

                The masked mul + d-reduce run in two k-halves so the reduce
                of half 0 overlaps the mul of half 1 in the DVE pipe.
                """
                sBt = T([BJ, KD, KT], f"sBt_{m}")
                s_sb = T([BJ, KD], f"s_sb_{m}")
                nc.vector.tensor_tensor(
                    sBt[:, :, :],
                    y_ps[:, :].unsqueeze(1).broadcast_to((BJ, KD, KT)),
                    WrBIGt[:, :, :, :].rearrange("p k b d -> p k (b d)"),
                    ALU.mult,
                )
                nc.vector.reduce_sum(
                    s_sb[:, :], sBt[:, :, :], axis=mybir.AxisListType.X
                )
                s2 = T([BJ, KD], f"s2_{m}")
                nc.vector.tensor_tensor(s2[:, :], s_sb[:, :], s_sb[:, :], ALU.mult)
                nsq_ps = yp.tile([BJ, KD], f32, tag="ypsum", name=f"nsq_{m}")
                nc.tensor.matmul(
                    nsq_ps[:, :], cB80, s2[:, :], start=True, stop=True
                )
                nsq_sb = T([BJ, KD], f"nsq_sb_{m}")
                nc.vector.tensor_scalar_add(nsq_sb[:, :], nsq_ps[:, :], 1e-12)
                u = T([BJ, KD], f"u_{m}")
                nc.gpsimd.tensor_tensor(
                    u[:, :], nsq_sb[:, :], halfs[:, :], ALU.pow
                )
                dd = T([BJ, KD], f"dd_{m}")
                nc.vector.scalar_tensor_tensor(
                    dd[:, :], nsq_sb[:, :], 1.0, u[:, :], ALU.add, ALU.mult
                )
                rr = T([BJ, KD], f"rr_{m}")
                nc.vector.reciprocal_approx_fast(rr[:, :], dd[:, :])
                sc = T([BJ, KD], f"sc_{m}")
                nc.vector.tensor_tensor(sc[:, :], nsq_sb[:, :], rr[:, :], ALU.mult)
                o_sb = T([BJ, KD], f"o_{m}")
                nc.vector.tensor_tensor(o_sb[:, :], s_sb[:, :], sc[:, :], ALU.mult)
                return o_sb

            def vhat_update(m, o_sb):
                """vacc (+)= Wr . o; build blkv (hi/lo bf16)."""
                vt = T([BJ, DA, KD], f"vt_{m}")
                nc.vector.tensor_tensor(
                    vt[:, :, :],
                    o_sb[:, :].unsqueeze(1).broadcast_to((BJ, DA, KD)),
                    Wr[:, :, :], ALU.mult,
                )
                if m == 0:
                    nc.vector.reduce_sum(
                        vacc[:, :], vt[:, :, :], axis=mybir.AxisListType.X
                    )
                else:
                    v_cur = T([BJ, DA], f"v_cur_{m}")
                    nc.vector.reduce_sum(
                        v_cur[:, :], vt[:, :, :], axis=mybir.AxisListType.X
                    )
                    nc.vector.tensor_tensor(
                        vacc[:, :], vacc[:, :], v_cur[:, :], ALU.add
                    )
                vT_ps = yp.tile([DA, BJ], f32, tag="ypsum", name=f"vT_{m}")
                nc.tensor.transpose(vT_ps[:, :], vacc[:, :], cI80)
                vT_sb = T([DA, BJ], f"vT_sb_{m}")
                nc.vector.tensor_copy(vT_sb[:, :], vT_ps[:, :])
                vdup_ps = yp.tile([KT, BJ], f32, tag="ypsum", name=f"vd_{m}")
                nc.tensor.matmul(
                    vdup_ps[:, :], cREP, vT_sb[:, :], start=True, stop=True
                )
                blkM = T([KT, BJ], f"blkM_{m}")
                nc.vector.tensor_tensor(
                    blkM[:, :], cBLK, vdup_ps[:, :], ALU.mult
                )
                blkv_n = blkv_t[m]
                nc.vector.tensor_copy(blkv_n[:, 0, :], blkM[:, :])
                nc.gpsimd.tensor_sub(
                    blkv_n[:, 1, :], blkM[:, :], blkv_n[:, 0, :]
                )
                if _DEBUG_TAPS:
                    nc.sync.dma_start(dbg_d[m], vacc[:, :])

            # ================= m = 0 (uniform c shortcut, f32) =================
            y0r_ps = yp.tile([1, NB * D], f32, tag="ypsum", name="y0r")
            nc.tensor.matmul(
                y0r_ps[:, :], onesF[:, :], part0[:, :, :], start=True, stop=True
            )
            nc.vector.tensor_copy(
                y0row[:, :].rearrange("p (b d) -> p b d", d=DA)[:, :, 0:D],
                y0r_ps[:, :].rearrange("p (b d) -> p b d", d=D),
            )
            nc.vector.memset(
                y0row[:, :].rearrange("p (b d) -> p b d", d=DA)[:, :, D],
                float(IN),
            )
            # WrBIGt[(j,b), k, (b',d)] = cBLKY * Wr (emitted after the y0row
            # ops so the m0 chain isn't queued behind it on DVE)
            nc.vector.tensor_tensor(
                WrBIGt[:, :, :, :],
                cBLKY.rearrange("p (b d) -> p b d", d=DA)
                .unsqueeze(1).broadcast_to((BJ, KD, NB, DA)),
                Wr[:, :, :].transpose([0, 2, 1])
                .unsqueeze(2).broadcast_to((BJ, KD, NB, DA)),
                ALU.mult,
            )
            y_ps0 = yp.tile([BJ, KT], f32, tag="ypsum", name="y_0")
            nc.tensor.matmul(
                y_ps0[:, :], tenth80[:, :], y0row[:, :], start=True, stop=True
            )
            if _DEBUG_TAPS:
                nc.sync.dma_start(dbg2_d[0, :, 0:DA * KD], Wr[:, :, :])
                nc.sync.dma_start(
                    dbg2_d[1],
                    WrBIGt[:, :, :, :].rearrange("p k b d -> p (k b d)"),
                )
                ydbg = T([BJ, KT], "ydbg")
                nc.vector.tensor_copy(ydbg[:, :], y_ps0[:, :])
                nc.sync.dma_start(dbg2_d[2, :, 0:KT], ydbg[:, :])
                nc.sync.dma_start(
                    dbg2_d[3, :, 0:CPACK_W], cpack[0:BJ, :]
                )
            o0 = s_and_squash(0, y_ps0)
            vhat_update(0, o0)

            # ----- xT build: 64 PE transposes, copies in 8-chunk groups -----
            with tc.tile_pool(name="tpp", bufs=4, space="PSUM") as tpp:
                for w in range(0, NH, 8):
                    tp = tpp.tile([KT, 8, P], bf16, tag="tp", name=f"tp_{w}")
                    for q in range(8):
                        nc.tensor.transpose(
                            tp[:, q, :], x_bf[:, w + q, :, :], cIbf[:, :]
                        )
                    if (w // 8) % 2 == 1:
                        nc.scalar.copy(xT[:, w:w + 8, :], tp[:, :, :])
                    else:
                        nc.vector.tensor_copy(xT[:, w:w + 8, :], tp[:, :, :])

            # ================= m = 1, 2 =================
            with tc.tile_pool(name="bwp", bufs=3, space="PSUM") as bwp:
                for m in (1, 2):
                    blkv = blkv_t[m - 1]
                    e = e_st[m]
                    cst = c_st[m]
                    u5, v2, w1 = u5_t[m], v2_t[m], w1_t[m]
                    Z, Zr, Zrb = Z_t[m], Zr_t[m], Zrb_t[m]

                    y_ps = yp.tile([BJ, KT], f32, tag="ypsum", name=f"y_{m}")

                    for h0, QH in ((0, 24), (24, 24), (48, 8), (56, 8)):
                        # --- b-logit waves: 8 (or 4) chunks each ---
                        for w0 in range(0, QH, 8):
                            wn = min(8, QH - w0)
                            wh = wn // 2
                            bw = bwp.tile(
                                [P, 2, 512], f32,
                                tag="bw", name=f"bw_{m}_{h0}_{w0}",
                            )
                            for c in range(wn):
                                h = h0 + w0 + c
                                off = (c % wh) * BJ
                                dst = bw[:, c // wh, off:off + BJ]
                                nc.tensor.matmul(
                                    dst, xT[:, h, :], blkv[:, 0, :],
                                    start=True, stop=False,
                                )
                                nc.tensor.matmul(
                                    dst, xT[:, h, :], blkv[:, 1, :],
                                    start=False, stop=True,
                                )
                            # exp -> e[p, h, j, b] (h-outer, contiguous)
                            hw0 = h0 + w0
                            nc.scalar.activation(
                                e[:, hw0:hw0 + wn, :, :]
                                .rearrange("p (a c) j b -> p a c (j b)", a=2),
                                bw[:, :, 0:wh * BJ]
                                .rearrange("p a (c x) -> p a c x", x=BJ),
                                AF.Exp,
                            )
                        hs = slice(h0, h0 + QH)
                        # --- Z = sum_j e via bf16 pair tree (DVE 2x) ---
                        nc.vector.tensor_tensor(
                            u5[:, hs, :, :], e[:, hs, 0:5, :], e[:, hs, 5:10, :],
                            ALU.add,
                        )
                        nc.vector.tensor_tensor(
                            v2[:, hs, :, :], u5[:, hs, 0:2, :], u5[:, hs, 2:4, :],
                            ALU.add,
                        )
                        nc.vector.tensor_tensor(
                            w1[:, hs, :], v2[:, hs, 0, :], v2[:, hs, 1, :],
                            ALU.add,
                        )
                        nc.vector.tensor_tensor(
                            Z[:, hs, :], w1[:, hs, :], u5[:, hs, 4, :], ALU.add
                        )
                        nc.vector.reciprocal_approx_fast(
                            Zr[:, hs, :].rearrange("p h b -> p (h b)"),
                            Z[:, hs, :].rearrange("p h b -> p (h b)"),
                        )
                        nc.vector.tensor_copy(Zrb[:, hs, :], Zr[:, hs, :])
                        # --- c = e * Zr (outer-dim broadcast keeps DVE 2x);
                        # j 0:6 on DVE, 6:10 on Pool ---
                        nc.vector.tensor_tensor(
                            cst[:, hs, 0:6, :], e[:, hs, 0:6, :],
                            Zrb[:, hs, :].unsqueeze(2)
                            .broadcast_to((P, QH, 6, NB)),
                            ALU.mult,
                        )
                        nc.gpsimd.tensor_mul(
                            cst[:, hs, 6:10, :], e[:, hs, 6:10, :],
                            Zrb[:, hs, :].unsqueeze(2)
                            .broadcast_to((P, QH, 4, NB)),
                        )
                        # --- y accumulation for this quarter ---
                        for h in range(h0, h0 + QH):
                            nc.tensor.matmul(
                                y_ps[:, :],
                                cst[:, h, :, :],
                                x_bf[:, h, :, :],
                                start=(h == 0), stop=(h == NH - 1),
                            )

                    o_sb = s_and_squash(m, y_ps)

                    if m < 2:
                        vhat_update(m, o_sb)
                    else:
                        # ---- final lengths ||o[(j,b), :]|| over k ----
                        osq = T([BJ, KD], "osq")
                        lsum = T([BJ, 1], "lsum")
                        nc.vector.tensor_tensor(
                            osq[:, :], o_sb[:, :], o_sb[:, :], ALU.mult
                        )
                        nc.vector.reduce_sum(
                            lsum[:, :], osq[:, :], axis=mybir.AxisListType.X
                        )
                        lnorm = T([BJ, 1], "lnorm")
                        nc.gpsimd.tensor_tensor(
                            lnorm[:, :], lsum[:, :], halfs[:, 0:1], ALU.pow
                        )
                        nc.sync.dma_start(out_d[:, :], lnorm[:, :])

    nc.compile()
    return nc


_NC_CACHE = None


def _get_nc():
    global _NC_CACHE
    if _NC_CACHE is None:
        _NC_CACHE = _build_nc()
    return _NC_CACHE


def kernel(x, W, bias):
    x = np.ascontiguousarray(np.asarray(x, dtype=np.float32))
    W = np.ascontiguousarray(np.asarray(W, dtype=np.float32))
    bias = np.ascontiguousarray(np.asarray(bias, dtype=np.float32))
    B = x.shape[0]
    per = B // N_CORES

    nc = _get_nc()
    in_maps = [
        {"x": x[i * per:(i + 1) * per], "W": W, "bias": bias}
        for i in range(N_CORES)
    ]
    res = bass_utils.run_bass_kernel_spmd(
        nc, in_maps, core_ids=list(range(N_CORES))
    )
    # rows are (j, b): out[j*8+b] -> [b, j]
    outs = [r["out"].reshape(J, NB).T for r in res.results]
    return np.concatenate(outs, axis=0)


if __name__ == "__main__":
    rng = np.random.default_rng(0)
    x = rng.standard_normal((64, IN, D), dtype=np.float32)
    W = (rng.standard_normal((D, J * KD)) / np.sqrt(D)).astype(np.float32)
    bias = (rng.standard_normal(J * KD) * 0.01).astype(np.float32)
    out = kernel(x=x, W=W, bias=bias)
    print(out.shape, out[0])



# revision 21
# speedup vs baseline: 1.1193x; 1.1193x over previous
"""DenseCapsule routing kernel for Trainium2 (Bass/Tile), 8-core data-parallel.

Problem: x [64, 8192, 8], W [8, 160], bias [160] ->
  x_hat = (x @ W + bias).reshape(64, 8192, 10, 16)
  3 dynamic-routing iterations (softmax over out_num=10, weighted sum over
  in_num=8192, squash over the 10-axis, agreement update), return
  ||outputs||_2 over out_dim -> [64, 10].

Key algebra (x_hat never materialized):
  yT[(b,d), (j,b')] = sum_i x_aug[i,(b,d)] c[i,(j,b')]   (PE, masked by cBLK)
  s8T[k, (j,b)]     = per-j matmuls W_aug vs masked yT    (PE, f32)
  squash runs on the k-partition layout [16, (j,b)] so the vhat matmuls
  need no transposes; vT[d, (j,b)] via per-j matmuls; blkv = mask(cREP@vacc).
  b_logits = xT^T @ blkv, single fp16 blkv (no hi/lo split).
  softmax: exp on ACT (bf16), Z pair-tree on DVE 2x, c = e*Zr -> fp16.

Sharding: batch 64 -> 8 cores x 8 batches. Row space (b,d) = b*9+d (72 rows),
cols (j,b) = j*8+b (80). Output row [1, 80] = lengths at (j,b).
"""

from contextlib import ExitStack

import numpy as np

import concourse.bacc as bacc
import concourse.bass as bass
import concourse.mybir as mybir
import concourse.tile as tile
import concourse.bass_utils as bass_utils

f32 = mybir.dt.float32
bf16 = mybir.dt.bfloat16
fp16 = mybir.dt.float16
AF = mybir.ActivationFunctionType
ALU = mybir.AluOpType

P = 128          # SBUF partitions
NH = 64          # i-chunks per batch (8192 / 128)
NB = 8           # batches per core
D = 8            # input capsule dim
DA = 9           # augmented (+ ones column)
J = 10           # out_num
KD = 16          # out_dim
KT = NB * DA     # 72 rows (b, d)
BJ = NB * J      # 80 cols (j, b) = j*8+b
IN = 8192
N_CORES = 8


def _build_nc():
    nc = bacc.Bacc(
        "TRN2", target_bir_lowering=False, debug=False, num_devices=N_CORES
    )

    x_d = nc.dram_tensor("x", [NB, IN, D], f32, kind="ExternalInput").ap()
    w_d = nc.dram_tensor("W", [D, J * KD], f32, kind="ExternalInput").ap()
    bias_d = nc.dram_tensor("bias", [J * KD], f32, kind="ExternalInput").ap()
    out_d = nc.dram_tensor("out", [1, BJ], f32, kind="ExternalOutput").ap()

    # ---- structural constants ----
    # cpack cols: 0:80 cBLK (rows 0:72), 80:152 cREP (rows 0:9),
    #             152:161 eye9 (rows 0:9), 161:162 ones column (all rows)
    C_BLK, C_REP, C_E9, C_ONE = 0, 80, 152, 161
    CPW = C_ONE + 1
    cpack_np = np.zeros((P, CPW), dtype=np.float32)
    for b in range(NB):
        for d in range(DA):
            for j in range(J):
                cpack_np[b * DA + d, C_BLK + j * NB + b] = 1.0
    for d in range(DA):
        for b in range(NB):
            cpack_np[d, C_REP + b * DA + d] = 1.0
    cpack_np[0:DA, C_E9:C_E9 + DA] = np.eye(DA, dtype=np.float32)
    cpack_np[:, C_ONE] = 1.0

    identf16_np = np.eye(P, dtype=np.float16)

    cpack_d = nc.inline_tensor(cpack_np, "cpack").ap()
    identf16_d = nc.inline_tensor(identf16_np, "identf16").ap()

    with tile.TileContext(nc) as tc, ExitStack() as ctx:
        sbp = ctx.enter_context(tc.tile_pool(name="sbp", bufs=1))

        def T(shape, name, dt=f32):
            return sbp.tile(shape, dt, name=name, tag=name)

        # ----- persistent SBUF tensors -----
        x_main = T([P, NB, NH, D], "x_main")          # raw DMA staging
        x_f16 = T([P, NH, KT], "x_f16", fp16)         # fp16 x_aug
        xT = T([KT, NH, P], "xT", fp16)               # x_aug^T per chunk
        cpack = T([P, CPW], "cpack")
        identf16 = T([P, P], "identf16", fp16)
        cBLK = cpack[0:KT, C_BLK:C_BLK + BJ]
        cREP = cpack[0:DA, C_REP:C_REP + KT]
        eye9 = cpack[0:DA, C_E9:C_E9 + DA]
        onesF = cpack[:, C_ONE:C_ONE + 1]

        W10flat = T([DA, J * KD], "W10flat")          # W_aug rows d
        WBIGall = T([KT, J * KD], "WBIGall")          # W_aug repl. over b
        WT10 = T([KD, J, DA], "WT10")                 # W_aug^T per j
        part0 = T([P, NB, D], "part0")                # m0 f32 colsum partials
        y0row = T([1, KT], "y0row")                   # m0 colsum row
        tenth80 = T([1, BJ], "tenth80")               # 0.1 expander row
        mh16 = T([KD, NB], "mh16")                    # -0.5 (pow -> rsqrt)
        halfrow = T([1, BJ], "halfrow")               # +0.5 (final sqrt)
        vaccT = T([DA, BJ], "vaccT")                  # accumulated vhat^T
        blkv_t = [T([KT, BJ], f"blkv{m}", fp16) for m in range(2)]
        lsum = T([1, BJ], "lsum")
        lenrow = T([1, BJ], "lenrow")
        powwarm = T([1, 1], "powwarm")

        e_st = [None, T([P, NH, J, NB], "e1", bf16), T([P, NH, J, NB], "e2", bf16)]
        c_st = [None, T([P, NH, J, NB], "c1", fp16), T([P, NH, J, NB], "c2", fp16)]
        u5_t = [None, T([P, NH, 5, NB], "u5_1", bf16), T([P, NH, 5, NB], "u5_2", bf16)]
        v2_t = [None, T([P, NH, 2, NB], "v2_1", bf16), T([P, NH, 2, NB], "v2_2", bf16)]
        w1_t = [None, T([P, NH, NB], "w1_1", bf16), T([P, NH, NB], "w1_2", bf16)]
        Z_t = [None, T([P, NH, NB], "Z_1"), T([P, NH, NB], "Z_2")]
        Zr_t = [None, T([P, NH, NB], "Zr_1"), T([P, NH, NB], "Zr_2")]
        Zrb_t = [None, T([P, NH, NB], "Zrb_1", bf16), T([P, NH, NB], "Zrb_2", bf16)]

        # per-m tail tensors
        yTm_t = [T([KT, BJ], f"yTm_{m}") for m in range(3)]
        s8T_t = [T([KD, J, NB], f"s8T_{m}") for m in range(3)]
        s2T_t = [T([KD, J, NB], f"s2T_{m}") for m in range(3)]
        nsq_t = [T([KD, NB], f"nsq_{m}") for m in range(3)]
        uin_t = [T([KD, NB], f"uin_{m}") for m in range(3)]
        dd_t = [T([KD, NB], f"dd_{m}") for m in range(3)]
        sc_t = [T([KD, NB], f"sc_{m}") for m in range(3)]
        o8T_t = [T([KD, J, NB], f"o8T_{m}") for m in range(3)]
        osqT = T([KD, J, NB], "osqT")

        # ----- input x: b0-b4 via HWDGE queues, b5-b7 via gpsimd SWDGE -----
        def xdma(eng, b):
            eng.dma_start(
                x_main[:, b, :, :],
                x_d[b].rearrange("(p h) d -> p h d", p=P),
            )

        xdma(nc.sync, 0)
        nc.sync.dma_start(identf16[:, :], identf16_d[:, :])
        xdma(nc.sync, 1)
        xdma(nc.sync, 2)
        xdma(nc.sync, 3)
        xdma(nc.sync, 4)
        nc.sync.dma_start(cpack[:, :], cpack_d[:, :])
        nc.sync.dma_start(W10flat[0:D, :], w_d[:, :])
        nc.sync.dma_start(
            W10flat[D:DA, :],
            bass.AP(tensor=bias_d.tensor, offset=0,
                    ap=[[J * KD, 1], [1, J * KD]]),
        )
        for b in (5, 6, 7):
            xdma(nc.gpsimd, b)

        # tiny memsets on gpsimd; pad memset on DVE (runs during first DMAs)
        nc.gpsimd.memset(mh16[:, :], -0.5)
        nc.gpsimd.memset(halfrow[:, :], 0.5)
        nc.gpsimd.memset(tenth80[:, :], 1.0 / J)
        nc.vector.memset(
            x_f16[:, :, :].rearrange("p h (b d) -> p h b d", d=DA)[:, :, :, D],
            1.0,
        )
        # warm the gpsimd pow library off the critical chain
        nc.gpsimd.tensor_tensor(
            powwarm[:, :], halfrow[0:1, 0:1], halfrow[0:1, 0:1], ALU.pow
        )

        # ----- per-batch cast (ACT) + m0 colsum (DVE) -----
        for b in range(NB):
            dst = x_f16[:, :, b * DA:b * DA + D]
            nc.scalar.copy(dst, x_main[:, b, :, :])
            nc.vector.reduce_sum(
                part0[:, b, :],
                x_main[:, b, :, :].transpose([0, 2, 1]),
                axis=mybir.AxisListType.X,
            )

        with tc.tile_pool(name="wpp", bufs=1, space="PSUM") as wpp:
            # ---- W prep: WBIGall + WT10 (early, overlaps x DMA) ----
            wb_ps = wpp.tile([KT, J * KD], f32, tag="wb", name="wb_ps")
            nc.tensor.matmul(
                wb_ps[:, :], cREP, W10flat[:, :], start=True, stop=True
            )
            nc.vector.tensor_copy(WBIGall[:, :], wb_ps[:, :])
            wt_ps = wpp.tile([KD, J, DA], f32, tag="wt", name="wt_ps")
            for j in range(J):
                nc.tensor.transpose(
                    wt_ps[:, j, :], W10flat[:, j * KD:(j + 1) * KD], eye9
                )
            nc.vector.tensor_copy(WT10[:, :, :], wt_ps[:, :, :])

            # ---- PE warmup: dummy transposes gated on late-batch casts ----
            warm_ps = wpp.tile([DA, 4, P], fp16, tag="warm", name="warm_ps")
            for b in (3, 4, 5, 6):
                for r in range(9):
                    nc.tensor.transpose(
                        warm_ps[:, r % 4, :],
                        x_f16[:, r, b * DA:b * DA + DA],
                        identf16[:, :],
                    )

        with tc.tile_pool(name="ypp", bufs=1, space="PSUM") as ypp, \
             tc.tile_pool(name="tpp", bufs=3, space="PSUM") as tpp:

            def tail(m, yT72, fill=()):
                """mask -> s-MMs -> squash (k-layout) -> v-MMs -> blkv.

                fill: callbacks emitting PE work injected after the s-MM /
                v-MM stages so PE stays busy during the DVE scalar chain.
                """
                yTm, s8T, s2T = yTm_t[m], s8T_t[m], s2T_t[m]
                nsqT, uinv, ddT, scT = nsq_t[m], uin_t[m], dd_t[m], sc_t[m]
                o8T = o8T_t[m]
                # mask: yTm[(b,d),(j,b')] = yT * (b==b')
                nc.vector.tensor_tensor(yTm[:, :], yT72, cBLK, ALU.mult)
                s8_ps = ypp.tile([KD, J, NB], f32, tag="ysm", name=f"s8_{m}")
                for j in range(J):
                    nc.tensor.matmul(
                        s8_ps[:, j, :],
                        WBIGall[:, j * KD:(j + 1) * KD],
                        yTm[:, j * NB:(j + 1) * NB],
                        start=True, stop=True,
                    )

                # squash scalars on [16, 8] (k-partition layout)
                nc.vector.tensor_copy(s8T[:, :, :], s8_ps[:, :, :])
                nc.scalar.activation(s2T[:, :, :], s8_ps[:, :, :], AF.Square)
                nc.vector.reduce_sum(
                    nsqT[:, :],
                    s2T[:, :, :].transpose([0, 2, 1]),
                    axis=mybir.AxisListType.X,
                )
                nc.vector.tensor_scalar_add(nsqT[:, :], nsqT[:, :], 1e-12)
                nc.gpsimd.tensor_tensor(
                    uinv[:, :], nsqT[:, :], mh16[:, :], ALU.pow
                )
                if len(fill) > 0:
                    fill[0]()
                nc.vector.scalar_tensor_tensor(
                    ddT[:, :], nsqT[:, :], 1.0, uinv[:, :], ALU.add, ALU.mult
                )
                nc.vector.reciprocal_approx_fast(scT[:, :], ddT[:, :])
                # o8T = s8T * sc  (sc broadcast over j)
                nc.vector.tensor_tensor(
                    o8T[:, :, :],
                    s8T[:, :, :],
                    scT[:, :].unsqueeze(1).broadcast_to((KD, J, NB)),
                    ALU.mult,
                )
                if m == 2:
                    # final lengths: ||o||_k per (j,b) via ones-matmul
                    nc.scalar.activation(
                        osqT[:, :, :], o8T[:, :, :], AF.Square
                    )
                    ls_ps = ypp.tile([1, BJ], f32, tag="ysm", name="ls_ps")
                    nc.tensor.matmul(
                        ls_ps[:, :], onesF[0:KD, :],
                        osqT[:, :, :].rearrange("p j b -> p (j b)"),
                        start=True, stop=True,
                    )
                    nc.vector.tensor_copy(lsum[:, :], ls_ps[:, :])
                    nc.gpsimd.tensor_tensor(
                        lenrow[:, :], lsum[:, :], halfrow[:, :], ALU.pow
                    )
                    nc.sync.dma_start(out_d[:, :], lenrow[:, :])
                    return
                # vhat: vT[d,(j,b)] via per-j matmuls; accumulate; expand+mask
                vT_ps = ypp.tile([DA, BJ], f32, tag="ysm", name=f"vT_{m}")
                for j in range(J):
                    nc.tensor.matmul(
                        vT_ps[:, j * NB:(j + 1) * NB],
                        WT10[:, j, :],
                        o8T[:, j, :],
                        start=True, stop=True,
                    )
                if len(fill) > 1:
                    fill[1]()
                if m == 0:
                    nc.vector.tensor_copy(vaccT[:, :], vT_ps[:, :])
                else:
                    nc.vector.tensor_tensor(
                        vaccT[:, :], vaccT[:, :], vT_ps[:, :], ALU.add
                    )
                vd_ps = ypp.tile([KT, BJ], f32, tag="ysm", name=f"vd_{m}")
                nc.tensor.matmul(
                    vd_ps[:, :], cREP, vaccT[:, :], start=True, stop=True
                )
                nc.vector.tensor_tensor(
                    blkv_t[m][:, :], vd_ps[:, :], cBLK, ALU.mult
                )

            # ================= m = 0 (uniform c shortcut, f32) =================
            y0r_ps = ypp.tile([1, NB * D], f32, tag="ysm", name="y0r")
            nc.tensor.matmul(
                y0r_ps[:, :], onesF, part0[:, :, :], start=True, stop=True
            )
            nc.vector.tensor_copy(
                y0row[:, :].rearrange("p (b d) -> p b d", d=DA)[:, :, 0:D],
                y0r_ps[:, :].rearrange("p (b d) -> p b d", d=D),
            )
            nc.vector.memset(
                y0row[:, :].rearrange("p (b d) -> p b d", d=DA)[:, :, D],
                float(IN),
            )
            yT0_ps = ypp.tile([KT, BJ], f32, tag="ysm", name="yT0")
            nc.tensor.matmul(
                yT0_ps[:, :], y0row[:, :], tenth80[:, :], start=True, stop=True
            )

            # ----- xT build: 64 PE transposes + PSUM->SBUF engine copies -----
            tp_tiles = {}

            def emit_xt_tp(w):
                tp = tpp.tile([KT, 8, P], fp16, tag="tp", name=f"tp_{w}")
                for q in range(8):
                    nc.tensor.transpose(
                        tp[:, q, :], x_f16[:, w + q, :], identf16[:, :]
                    )
                tp_tiles[w] = tp

            def emit_xt_copy(w, eng):
                dst = xT[:, w:w + 8, :]
                tp = tp_tiles[w]
                if eng == "act":
                    nc.scalar.copy(dst, tp[:, :, :])
                else:
                    nc.vector.tensor_copy(dst, tp[:, :, :])

            emit_xt_tp(0)
            emit_xt_copy(0, "act")
            emit_xt_tp(8)
            tail(
                0, yT0_ps[:, :],
                fill=(
                    lambda: (emit_xt_tp(16), emit_xt_tp(24)),
                    lambda: (emit_xt_tp(32), emit_xt_tp(40)),
                ),
            )
            emit_xt_copy(8, "act")
            for w in (16, 24, 32, 40):
                emit_xt_copy(w, "dve")

            # ================= m = 1, 2 =================
            with tc.tile_pool(name="bwp", bufs=2, space="PSUM") as bwp:
                for m in (1, 2):
                    blkv = blkv_t[m - 1]
                    e = e_st[m]
                    cst = c_st[m]
                    u5, v2, w1 = u5_t[m], v2_t[m], w1_t[m]
                    Z, Zr, Zrb = Z_t[m], Zr_t[m], Zrb_t[m]

                    yT_ps = ypp.tile([KT, BJ], f32, tag="ysm", name=f"yT_{m}")

                    def emit_logits_exp(g):
                        # 2 waves of 8 chunks; exp per wave
                        for w in range(2):
                            hw0 = g * 16 + w * 8
                            bw = bwp.tile(
                                [P, 2, 512], f32, tag="bw",
                                name=f"bw_{m}_{hw0}",
                            )
                            for c8 in range(8):
                                h = hw0 + c8
                                off = (c8 % 4) * BJ
                                nc.tensor.matmul(
                                    bw[:, c8 // 4, off:off + BJ],
                                    xT[:, h, :], blkv[:, :],
                                    start=True, stop=True,
                                )
                            nc.scalar.activation(
                                e[:, hw0:hw0 + 8, :, :]
                                .rearrange("p (a c) j b -> p a c (j b)", a=2),
                                bw[:, :, 0:4 * BJ]
                                .rearrange("p a (c x) -> p a c x", x=BJ),
                                AF.Exp,
                            )

                    def emit_tree_c_y(g):
                        hs = slice(g * 16, g * 16 + 16)
                        QH = 16
                        nc.vector.tensor_tensor(
                            u5[:, hs, :, :], e[:, hs, 0:5, :], e[:, hs, 5:10, :],
                            ALU.add,
                        )
                        nc.vector.tensor_tensor(
                            v2[:, hs, :, :], u5[:, hs, 0:2, :], u5[:, hs, 2:4, :],
                            ALU.add,
                        )
                        nc.vector.tensor_tensor(
                            w1[:, hs, :], v2[:, hs, 0, :], v2[:, hs, 1, :],
                            ALU.add,
                        )
                        nc.vector.tensor_tensor(
                            Z[:, hs, :], w1[:, hs, :], u5[:, hs, 4, :], ALU.add
                        )
                        nc.vector.reciprocal_approx_fast(
                            Zr[:, hs, :].rearrange("p h b -> p (h b)"),
                            Z[:, hs, :].rearrange("p h b -> p (h b)"),
                        )
                        nc.vector.tensor_copy(Zrb[:, hs, :], Zr[:, hs, :])
                        # c = e * Zr -> fp16; j 0:7 on DVE, 7:10 on Pool
                        nc.vector.tensor_tensor(
                            cst[:, hs, 0:7, :], e[:, hs, 0:7, :],
                            Zrb[:, hs, :].unsqueeze(2)
                            .broadcast_to((P, QH, 7, NB)),
                            ALU.mult,
                        )
                        nc.gpsimd.tensor_mul(
                            cst[:, hs, 7:10, :], e[:, hs, 7:10, :],
                            Zrb[:, hs, :].unsqueeze(2)
                            .broadcast_to((P, QH, 3, NB)),
                        )
                        for h in range(g * 16, g * 16 + 16):
                            nc.tensor.matmul(
                                yT_ps[:, :],
                                x_f16[:, h, :],
                                cst[:, h, :, :],
                                start=(h == 0), stop=(h == NH - 1),
                            )

                    emit_logits_exp(0)
                    for g in range(4):
                        if g < 3:
                            emit_logits_exp(g + 1)
                        if m == 1 and g in (0, 1):
                            w = 48 if g == 0 else 56
                            emit_xt_tp(w)
                            emit_xt_copy(w, "act")
                        emit_tree_c_y(g)

                    tail(m, yT_ps[:, :])

    nc.compile()
    return nc


_NC_CACHE = None


def _get_nc():
    global _NC_CACHE
    if _NC_CACHE is None:
        _NC_CACHE = _build_nc()
    return _NC_CACHE


def kernel(x, W, bias):
    x = np.ascontiguousarray(np.asarray(x, dtype=np.float32))
    W = np.ascontiguousarray(np.asarray(W, dtype=np.float32))
    bias = np.ascontiguousarray(np.asarray(bias, dtype=np.float32))
    B = x.shape[0]
    per = B // N_CORES

    nc = _get_nc()
    in_maps = [
        {"x": x[i * per:(i + 1) * per], "W": W, "bias": bias}
        for i in range(N_CORES)
    ]
    res = bass_utils.run_bass_kernel_spmd(
        nc, in_maps, core_ids=list(range(N_CORES))
    )
    # out row is (j, b): [1, 80] -> [b, j]
    outs = [r["out"].reshape(J, NB).T for r in res.results]
    return np.concatenate(outs, axis=0)


if __name__ == "__main__":
    rng = np.random.default_rng(0)
    x = rng.standard_normal((64, IN, D), dtype=np.float32)
    W = (rng.standard_normal((D, J * KD)) / np.sqrt(D)).astype(np.float32)
    bias = (rng.standard_normal(J * KD) * 0.01).astype(np.float32)
    out = kernel(x=x, W=W, bias=bias)
    print(out.shape, out[0])


# revision 35
# speedup vs baseline: 1.2557x; 1.1219x over previous
"""DenseCapsule routing kernel for Trainium2 (Bass/Tile), 8-core data-parallel.

Problem: x [64, 8192, 8], W [8, 160], bias [160] ->
  x_hat = (x @ W + bias).reshape(64, 8192, 10, 16)
  3 dynamic-routing iterations (softmax over out_num=10, weighted sum over
  in_num=8192, squash over the 10-axis, agreement update), return
  ||outputs||_2 over out_dim -> [64, 10].

Key algebra (x_hat never materialized):
  yT[(b,d), (j,b')] = sum_i x_aug[i,(b,d)] c[i,(j,b')]   (PE, masked by cBLK)
  s8T[k, (j,b)]     = per-j matmuls W_aug vs masked yT    (PE, f32)
  squash runs on the k-partition layout [16, (j,b)] so the vhat matmuls
  need no transposes; vT[d, (j,b)] via per-j matmuls; blkv = mask(cREP@vacc).
  b_logits = xT^T @ blkv, single fp16 blkv (no hi/lo split).
  softmax: exp on ACT (bf16), Z pair-tree on DVE 2x, c = e*Zr -> fp16.

Sharding: batch 64 -> 8 cores x 8 batches. Row space (b,d) = b*9+d (72 rows),
cols (j,b) = j*8+b (80). Output row [1, 80] = lengths at (j,b).
"""

from contextlib import ExitStack

import numpy as np

import concourse.bacc as bacc
import concourse.bass as bass
import concourse.mybir as mybir
import concourse.tile as tile
import concourse.bass_utils as bass_utils

f32 = mybir.dt.float32
bf16 = mybir.dt.bfloat16
fp16 = mybir.dt.float16
AF = mybir.ActivationFunctionType
ALU = mybir.AluOpType

P = 128          # SBUF partitions
NH = 64          # i-chunks per batch (8192 / 128)
NB = 8           # batches per core
D = 8            # input capsule dim
DA = 9           # augmented (+ ones column)
J = 10           # out_num
KD = 16          # out_dim
KT = NB * DA     # 72 rows (b, d)
BJ = NB * J      # 80 cols (j, b) = j*8+b
IN = 8192
N_CORES = 8


def _build_nc():
    nc = bacc.Bacc(
        "TRN2", target_bir_lowering=False, debug=False, num_devices=N_CORES
    )

    xf_d = nc.dram_tensor(
        "xf", [4, P, NH // 4, KT], fp16, kind="ExternalInput"
    ).ap()
    xt_d = nc.dram_tensor(
        "xt", [4, KT, NH // 4, P], fp16, kind="ExternalInput"
    ).ap()
    w_d = nc.dram_tensor("W", [D, J * KD], f32, kind="ExternalInput").ap()
    bias_d = nc.dram_tensor("bias", [J * KD], f32, kind="ExternalInput").ap()
    out_d = nc.dram_tensor("out", [1, BJ], f32, kind="ExternalOutput").ap()

    # ---- structural constants ----
    # cpack cols: 0:80 cBLK (rows 0:72), 80:152 cREP (rows 0:9),
    #             152:161 eye9 (rows 0:9), 161:162 ones column (all rows)
    C_BLK, C_REP, C_E9, C_ONE = 0, 80, 152, 161
    CPW = C_ONE + 1
    cpack_np = np.zeros((P, CPW), dtype=np.float32)
    for b in range(NB):
        for d in range(DA):
            for j in range(J):
                cpack_np[b * DA + d, C_BLK + j * NB + b] = 1.0
    for d in range(DA):
        for b in range(NB):
            cpack_np[d, C_REP + b * DA + d] = 1.0
    cpack_np[0:DA, C_E9:C_E9 + DA] = np.eye(DA, dtype=np.float32)
    cpack_np[:, C_ONE] = 1.0

    cpack_d = nc.inline_tensor(cpack_np, "cpack").ap()

    with tile.TileContext(nc) as tc, ExitStack() as ctx:
        sbp = ctx.enter_context(tc.tile_pool(name="sbp", bufs=1))

        def T(shape, name, dt=f32):
            return sbp.tile(shape, dt, name=name, tag=name)

        # ----- persistent SBUF tensors -----
        x_f16 = T([P, NH, KT], "x_f16", fp16)         # fp16 x_aug (host-cast)
        xT = T([KT, NH, P], "xT", fp16)               # x_aug^T (host-built)
        cpack = T([P, CPW], "cpack")
        cBLK = cpack[0:KT, C_BLK:C_BLK + BJ]
        cREP = cpack[0:DA, C_REP:C_REP + KT]
        eye9 = cpack[0:DA, C_E9:C_E9 + DA]
        onesF = cpack[:, C_ONE:C_ONE + 1]

        W10flat = T([DA, J * KD], "W10flat")          # W_aug rows d
        WBIGall = T([KT, J * KD], "WBIGall")          # W_aug repl. over b
        WT10 = T([KD, J, DA], "WT10")                 # W_aug^T per j
        part0 = T([P, 4, KT], "part0")                # m0 f32 colsum partials
        y0row = T([1, KT], "y0row")                   # m0 colsum row
        tenth80 = T([1, BJ], "tenth80")               # 0.1 expander row
        phalf16 = T([KD, NB], "phalf16")              # +0.5 (pow -> sqrt)
        halfrow = T([1, BJ], "halfrow")               # +0.5 (pow warm)
        vaccT = T([DA, BJ], "vaccT")                  # accumulated vhat^T
        blkv_t = [T([KT, BJ], f"blkv{m}", fp16) for m in range(2)]
        lsum = T([1, BJ], "lsum")
        powwarm = T([1, 1], "powwarm")

        e_st = [None, T([P, NH, J, NB], "e1", bf16), T([P, NH, J, NB], "e2", bf16)]
        c_st = [None, T([P, NH, J, NB], "c1", fp16), T([P, NH, J, NB], "c2", fp16)]
        u5_t = [None, T([P, NH, 5, NB], "u5_1", bf16), T([P, NH, 5, NB], "u5_2", bf16)]
        v2_t = [None, T([P, NH, 2, NB], "v2_1", bf16), T([P, NH, 2, NB], "v2_2", bf16)]
        w1_t = [None, T([P, NH, NB], "w1_1", bf16), T([P, NH, NB], "w1_2", bf16)]
        Z_t = [None, T([P, NH, NB], "Z_1"), T([P, NH, NB], "Z_2")]
        Zr_t = [None, T([P, NH, NB], "Zr_1"), T([P, NH, NB], "Zr_2")]
        Zrb_t = [None, T([P, NH, NB], "Zrb_1", bf16), T([P, NH, NB], "Zrb_2", bf16)]

        # per-m tail tensors
        yTm_t = [T([KT, BJ], f"yTm_{m}") for m in range(3)]
        s2T_t = [T([KD, J + 1, NB], f"s2T_{m}") for m in range(3)]
        nsq_t = [T([KD, NB], f"nsq_{m}") for m in range(3)]
        uin_t = [T([KD, NB], f"uin_{m}") for m in range(3)]
        wp1_t = [T([KD, NB], f"wp1_{m}") for m in range(3)]
        r1_t = [T([KD, NB], f"r1_{m}") for m in range(3)]
        sc_t = [T([KD, NB], f"sc_{m}") for m in range(3)]
        o8T_t = [T([KD, J, NB], f"o8T_{m}") for m in range(3)]
        osqT = T([KD, J, NB], "osqT")

        # ----- inputs: xf pieces first on sync HWDGE (feeds colsums);
        # consts + xt split over scalar HWDGE and gpsimd SWDGE ----------
        for k in range(4):
            nc.sync.dma_start(
                x_f16[:, k * (NH // 4):(k + 1) * (NH // 4), :], xf_d[k]
            )
        nc.scalar.dma_start(cpack[:, :], cpack_d[:, :])
        nc.scalar.dma_start(W10flat[0:D, :], w_d[:, :])
        nc.scalar.dma_start(
            W10flat[D:DA, :],
            bass.AP(tensor=bias_d.tensor, offset=0,
                    ap=[[J * KD, 1], [1, J * KD]]),
        )
        for k in (0, 1):
            nc.scalar.dma_start(
                xT[:, k * (NH // 4):(k + 1) * (NH // 4), :], xt_d[k]
            )
        for k in (2, 3):
            nc.gpsimd.dma_start(
                xT[:, k * (NH // 4):(k + 1) * (NH // 4), :], xt_d[k]
            )

        # tiny memsets on gpsimd; pad memset on DVE (runs during first DMAs)
        nc.gpsimd.memset(phalf16[:, :], 0.5)
        nc.gpsimd.memset(halfrow[:, :], 0.5)
        nc.gpsimd.memset(tenth80[:, :], 1.0 / J)
        for m in range(3):
            nc.gpsimd.memset(s2T_t[m][:, J, :], 1e-12)
        # warm the gpsimd pow library off the critical chain
        nc.gpsimd.tensor_tensor(
            powwarm[:, :], halfrow[0:1, 0:1], halfrow[0:1, 0:1], ALU.pow
        )

        # ----- m0 colsums (DVE): per-piece partials over h, then merge ----
        for k in range(4):
            nc.vector.reduce_sum(
                part0[:, k, :],
                x_f16[:, k * (NH // 4):(k + 1) * (NH // 4), :]
                .transpose([0, 2, 1]),
                axis=mybir.AxisListType.X,
            )

        with tc.tile_pool(name="wpp", bufs=1, space="PSUM") as wpp:
            # ---- W prep: WBIGall + WT10 (early, overlaps x DMA) ----
            wb_ps = wpp.tile([KT, J * KD], f32, tag="wb", name="wb_ps")
            nc.tensor.matmul(
                wb_ps[:, :], cREP, W10flat[:, :], start=True, stop=True
            )
            nc.scalar.copy(WBIGall[:, :], wb_ps[:, :])
            wt_ps = wpp.tile([KD, J, DA], f32, tag="wt", name="wt_ps")
            for j in range(J):
                nc.tensor.transpose(
                    wt_ps[:, j, :], W10flat[:, j * KD:(j + 1) * KD], eye9
                )
            nc.scalar.copy(WT10[:, :, :], wt_ps[:, :, :])



        with tc.tile_pool(name="ypp", bufs=1, space="PSUM") as ypp:

            def tail(m, yT72, fill=()):
                """mask -> s-MMs -> squash (k-layout) -> v-MMs -> blkv.

                fill: callbacks emitting PE work injected after the s-MM /
                v-MM stages so PE stays busy during the DVE scalar chain.
                """
                yTm, s2T = yTm_t[m], s2T_t[m]
                nsqT, u, wp1, r1, scT = (
                    nsq_t[m], uin_t[m], wp1_t[m], r1_t[m], sc_t[m]
                )
                o8T = o8T_t[m]
                # mask: yTm[(b,d),(j,b')] = yT * (b==b')
                nc.vector.tensor_tensor(yTm[:, :], yT72, cBLK, ALU.mult)
                s8_ps = ypp.tile([KD, J, NB], f32, tag="ysm", name=f"s8_{m}")
                for j in range(J):
                    nc.tensor.matmul(
                        s8_ps[:, j, :],
                        WBIGall[:, j * KD:(j + 1) * KD],
                        yTm[:, j * NB:(j + 1) * NB],
                        start=True, stop=True,
                    )

                # squash scalars on [16, 8] (k-partition layout);
                # eps lives in s2T's 11th j-column (prologue memset)
                nc.scalar.activation(s2T[:, 0:J, :], s8_ps[:, :, :], AF.Square)
                nc.vector.reduce_sum(
                    nsqT[:, :],
                    s2T[:, :, :].transpose([0, 2, 1]),
                    axis=mybir.AxisListType.X,
                )
                # sc = sqrt(nsq)/(1+nsq): pow on Pool overlaps the DVE recip
                nc.gpsimd.tensor_tensor(
                    u[:, :], nsqT[:, :], phalf16[:, :], ALU.pow
                )
                nc.vector.tensor_scalar_add(wp1[:, :], nsqT[:, :], 1.0)
                nc.vector.reciprocal_approx_fast(r1[:, :], wp1[:, :])
                if len(fill) > 0:
                    fill[0]()
                nc.vector.tensor_tensor(scT[:, :], u[:, :], r1[:, :], ALU.mult)
                # o8T = s8 * sc, straight from PSUM (sc broadcast over j)
                nc.vector.tensor_tensor(
                    o8T[:, :, :],
                    s8_ps[:, :, :],
                    scT[:, :].unsqueeze(1).broadcast_to((KD, J, NB)),
                    ALU.mult,
                )
                if m == 2:
                    # final lengths: ||o||_k per (j,b) via ones-matmul
                    nc.scalar.activation(
                        osqT[:, :, :], o8T[:, :, :], AF.Square
                    )
                    ls_ps = ypp.tile([1, BJ], f32, tag="ysm", name="ls_ps")
                    nc.tensor.matmul(
                        ls_ps[:, :], onesF[0:KD, :],
                        osqT[:, :, :].rearrange("p j b -> p (j b)"),
                        start=True, stop=True,
                    )
                    nc.vector.tensor_copy(lsum[:, :], ls_ps[:, :])
                    nc.sync.dma_start(out_d[:, :], lsum[:, :])
                    return
                # vhat: vT[d,(j,b)] via per-j matmuls; accumulate; expand+mask
                vT_ps = ypp.tile([DA, BJ], f32, tag="ysm", name=f"vT_{m}")
                for j in range(J):
                    nc.tensor.matmul(
                        vT_ps[:, j * NB:(j + 1) * NB],
                        WT10[:, j, :],
                        o8T[:, j, :],
                        start=True, stop=True,
                    )
                if len(fill) > 1:
                    fill[1]()
                if m == 0:
                    nc.vector.tensor_copy(vaccT[:, :], vT_ps[:, :])
                else:
                    nc.vector.tensor_tensor(
                        vaccT[:, :], vaccT[:, :], vT_ps[:, :], ALU.add
                    )
                vd_ps = ypp.tile([KT, BJ], f32, tag="ysm", name=f"vd_{m}")
                nc.tensor.matmul(
                    vd_ps[:, :], cREP, vaccT[:, :], start=True, stop=True
                )
                nc.vector.tensor_tensor(
                    blkv_t[m][:, :], vd_ps[:, :], cBLK, ALU.mult
                )

            # ================= m = 0 (uniform c shortcut, f32) =================
            # accumulate the 4 piece-partials in PSUM via 4 tiny matmuls
            y0r_ps = ypp.tile([1, KT], f32, tag="ysm", name="y0r")
            for k in range(4):
                nc.tensor.matmul(
                    y0r_ps[:, :], onesF, part0[:, k, :],
                    start=(k == 0), stop=(k == 3),
                )
            nc.vector.tensor_copy(y0row[:, :], y0r_ps[:, :])
            yT0_ps = ypp.tile([KT, BJ], f32, tag="ysm", name="yT0")
            nc.tensor.matmul(
                yT0_ps[:, :], y0row[:, :], tenth80[:, :], start=True, stop=True
            )

            tail(0, yT0_ps[:, :])

            # ================= m = 1, 2 =================
            with tc.tile_pool(name="bwp", bufs=3, space="PSUM") as bwp:
                for m in (1, 2):
                    blkv = blkv_t[m - 1]
                    e = e_st[m]
                    cst = c_st[m]
                    u5, v2, w1 = u5_t[m], v2_t[m], w1_t[m]
                    Z, Zr, Zrb = Z_t[m], Zr_t[m], Zrb_t[m]

                    yT_ps = ypp.tile([KT, BJ], f32, tag="ysm", name=f"yT_{m}")

                    def emit_logits_exp(g):
                        # 2 waves of 8 chunks; exp per wave
                        for w in range(2):
                            hw0 = g * 16 + w * 8
                            bw = bwp.tile(
                                [P, 2, 512], f32, tag="bw",
                                name=f"bw_{m}_{hw0}",
                            )
                            for c8 in range(8):
                                h = hw0 + c8
                                off = (c8 % 4) * BJ
                                nc.tensor.matmul(
                                    bw[:, c8 // 4, off:off + BJ],
                                    xT[:, h, :], blkv[:, :],
                                    start=True, stop=True,
                                )
                            nc.scalar.activation(
                                e[:, hw0:hw0 + 8, :, :]
                                .rearrange("p (a c) j b -> p a c (j b)", a=2),
                                bw[:, :, 0:4 * BJ]
                                .rearrange("p a (c x) -> p a c x", x=BJ),
                                AF.Exp,
                            )

                    def emit_tree_c_y(g):
                        hs = slice(g * 16, g * 16 + 16)
                        QH = 16
                        nc.vector.tensor_tensor(
                            u5[:, hs, :, :], e[:, hs, 0:5, :], e[:, hs, 5:10, :],
                            ALU.add,
                        )
                        nc.vector.tensor_tensor(
                            v2[:, hs, :, :], u5[:, hs, 0:2, :], u5[:, hs, 2:4, :],
                            ALU.add,
                        )
                        nc.vector.tensor_tensor(
                            w1[:, hs, :], v2[:, hs, 0, :], v2[:, hs, 1, :],
                            ALU.add,
                        )
                        nc.vector.tensor_tensor(
                            Z[:, hs, :], w1[:, hs, :], u5[:, hs, 4, :], ALU.add
                        )
                        nc.vector.reciprocal_approx_fast(
                            Zr[:, hs, :].rearrange("p h b -> p (h b)"),
                            Z[:, hs, :].rearrange("p h b -> p (h b)"),
                        )
                        nc.scalar.copy(Zrb[:, hs, :], Zr[:, hs, :])
                        # c = e * Zr -> fp16; j 0:7 on DVE, 7:10 on Pool
                        nc.vector.tensor_tensor(
                            cst[:, hs, 0:7, :], e[:, hs, 0:7, :],
                            Zrb[:, hs, :].unsqueeze(2)
                            .broadcast_to((P, QH, 7, NB)),
                            ALU.mult,
                        )
                        nc.gpsimd.tensor_mul(
                            cst[:, hs, 7:10, :], e[:, hs, 7:10, :],
                            Zrb[:, hs, :].unsqueeze(2)
                            .broadcast_to((P, QH, 3, NB)),
                        )
                        for h in range(g * 16, g * 16 + 16):
                            nc.tensor.matmul(
                                yT_ps[:, :],
                                x_f16[:, h, :],
                                cst[:, h, :, :],
                                start=(h == 0), stop=(h == NH - 1),
                            )

                    emit_logits_exp(0)
                    for g in range(4):
                        if g < 3:
                            emit_logits_exp(g + 1)
                        emit_tree_c_y(g)

                    tail(m, yT_ps[:, :])

    nc.compile()
    return nc


_NC_CACHE = None


def _get_nc():
    global _NC_CACHE
    if _NC_CACHE is None:
        _NC_CACHE = _build_nc()
    return _NC_CACHE


def kernel(x, W, bias):
    x = np.asarray(x, dtype=np.float32)
    W = np.ascontiguousarray(np.asarray(W, dtype=np.float32))
    bias = np.ascontiguousarray(np.asarray(bias, dtype=np.float32))
    B = x.shape[0]
    per = B // N_CORES

    nc = _get_nc()
    in_maps = _make_in_maps(x, W, bias)
    res = bass_utils.run_bass_kernel_spmd(
        nc, in_maps, core_ids=list(range(N_CORES))
    )
    # out row is (j, b) squared lengths: [1, 80] -> [b, j], sqrt here
    outs = [np.sqrt(r["out"]).reshape(J, NB).T for r in res.results]
    return np.concatenate(outs, axis=0)


def _make_in_maps(x, W, bias):
    B = x.shape[0]
    per = B // N_CORES
    in_maps = []
    for i in range(N_CORES):
        xc = x[i * per:(i + 1) * per]                       # [8, 8192, 8]
        xa = np.concatenate(
            [xc, np.ones((per, IN, 1), np.float32)], axis=2
        ).astype(np.float16)                                # [8, 8192, 9]
        r = xa.reshape(per, P, NH, DA)
        # xf[k, p, hh, (b,d)] = xa[b, p*NH + k*16 + hh, d]
        xf = np.ascontiguousarray(
            r.transpose(1, 2, 0, 3).reshape(P, 4, NH // 4, KT)
            .transpose(1, 0, 2, 3)
        )
        # xt[k, (b,d), h, p] = xa[b, p*NH + (k*16+h), d]
        xt_full = r.transpose(0, 3, 2, 1).reshape(KT, NH, P)
        xt = np.ascontiguousarray(xt_full.reshape(KT, 4, NH // 4, P)
                                  .transpose(1, 0, 2, 3))
        in_maps.append({"xf": xf, "xt": xt, "W": W, "bias": bias})
    return in_maps


if __name__ == "__main__":
    rng = np.random.default_rng(0)
    x = rng.standard_normal((64, IN, D), dtype=np.float32)
    W = (rng.standard_normal((D, J * KD)) / np.sqrt(D)).astype(np.float32)
    bias = (rng.standard_normal(J * KD) * 0.01).astype(np.float32)
    out = kernel(x=x, W=W, bias=bias)
    print(out.shape, out[0])


# revision 43
# speedup vs baseline: 1.2756x; 1.0158x over previous
"""DenseCapsule routing kernel for Trainium2 (Bass/Tile), 8-core data-parallel.

Problem: x [64, 8192, 8], W [8, 160], bias [160] ->
  x_hat = (x @ W + bias).reshape(64, 8192, 10, 16)
  3 dynamic-routing iterations (softmax over out_num=10, weighted sum over
  in_num=8192, squash over the 10-axis, agreement update), return
  ||outputs||_2 over out_dim -> [64, 10].

Key algebra (x_hat never materialized):
  yT[(b,d), (j,b')] = sum_i x_aug[i,(b,d)] c[i,(j,b')]   (PE, masked by cBLK)
  s8T[k, (j,b)]     = per-j matmuls W_aug vs masked yT    (PE, f32)
  squash runs on the k-partition layout [16, (j,b)] so the vhat matmuls
  need no transposes; vT[d, (j,b)] via per-j matmuls; blkv = mask(cREP@vacc).
  b_logits = xT^T @ blkv, single fp16 blkv (no hi/lo split).
  softmax: exp on ACT (bf16), Z pair-tree on DVE 2x, c = e*Zr -> fp16.

Sharding: batch 64 -> 8 cores x 8 batches. Row space (b,d) = b*9+d (72 rows),
cols (j,b) = j*8+b (80). Output row [1, 80] = lengths at (j,b).
"""

from contextlib import ExitStack

import numpy as np

import concourse.bacc as bacc
import concourse.bass as bass
import concourse.mybir as mybir
import concourse.tile as tile
import concourse.bass_utils as bass_utils

f32 = mybir.dt.float32
bf16 = mybir.dt.bfloat16
fp16 = mybir.dt.float16
AF = mybir.ActivationFunctionType
ALU = mybir.AluOpType

P = 128          # SBUF partitions
NH = 64          # i-chunks per batch (8192 / 128)
NB = 8           # batches per core
D = 8            # input capsule dim
DA = 9           # augmented (+ ones column)
J = 10           # out_num
KD = 16          # out_dim
KT = NB * DA     # 72 rows (b, d)
BJ = NB * J      # 80 cols (j, b) = j*8+b
IN = 8192
N_CORES = 8


def _build_nc():
    nc = bacc.Bacc(
        "TRN2", target_bir_lowering=False, debug=False, num_devices=N_CORES
    )

    xf_d = nc.dram_tensor(
        "xf", [4, P, NH // 4, KT], fp16, kind="ExternalInput"
    ).ap()
    xt_d = nc.dram_tensor(
        "xt", [4, KT, NH // 4, P], fp16, kind="ExternalInput"
    ).ap()
    w_d = nc.dram_tensor("W", [D, J * KD], f32, kind="ExternalInput").ap()
    bias_d = nc.dram_tensor("bias", [J * KD], f32, kind="ExternalInput").ap()
    out_d = nc.dram_tensor("out", [1, BJ], f32, kind="ExternalOutput").ap()

    # ---- structural constants ----
    # cpack cols: 0:80 cBLK (rows 0:72), 80:152 cREP (rows 0:9),
    #             152:161 eye9 (rows 0:9), 161:162 ones column (all rows)
    C_BLK, C_REP, C_E9, C_ONE = 0, 80, 152, 161
    CPW = C_ONE + 1
    cpack_np = np.zeros((P, CPW), dtype=np.float32)
    for b in range(NB):
        for d in range(DA):
            for j in range(J):
                cpack_np[b * DA + d, C_BLK + j * NB + b] = 1.0
    for d in range(DA):
        for b in range(NB):
            cpack_np[d, C_REP + b * DA + d] = 1.0
    cpack_np[0:DA, C_E9:C_E9 + DA] = np.eye(DA, dtype=np.float32)
    cpack_np[:, C_ONE] = 1.0

    cpack_d = nc.inline_tensor(cpack_np, "cpack").ap()

    with tile.TileContext(nc) as tc, ExitStack() as ctx:
        sbp = ctx.enter_context(tc.tile_pool(name="sbp", bufs=1))

        def T(shape, name, dt=f32):
            return sbp.tile(shape, dt, name=name, tag=name)

        # ----- persistent SBUF tensors -----
        x_f16 = T([P, NH, KT], "x_f16", fp16)         # fp16 x_aug (host-cast)
        xT = T([KT, NH, P], "xT", fp16)               # x_aug^T (host-built)
        cpack = T([P, CPW], "cpack")
        cBLK = cpack[0:KT, C_BLK:C_BLK + BJ]
        cREP = cpack[0:DA, C_REP:C_REP + KT]
        eye9 = cpack[0:DA, C_E9:C_E9 + DA]
        onesF = cpack[:, C_ONE:C_ONE + 1]

        W10flat = T([DA, J * KD], "W10flat")          # W_aug rows d
        WBIGall = T([KT, J * KD], "WBIGall")          # W_aug repl. over b
        WT10 = T([KD, J, DA], "WT10")                 # W_aug^T per j
        part0 = T([P, 4, KT], "part0")                # m0 f32 colsum partials
        y0row = T([1, KT], "y0row")                   # m0 colsum row
        tenth80 = T([1, BJ], "tenth80")               # 0.1 expander row
        phalf16 = T([KD, NB], "phalf16")              # +0.5 (pow -> sqrt)
        halfrow = T([1, BJ], "halfrow")               # +0.5 (pow warm)
        vaccT = T([DA, BJ], "vaccT")                  # accumulated vhat^T
        blkv_t = [T([KT, BJ], f"blkv{m}", fp16) for m in range(2)]
        lsum = T([1, BJ], "lsum")
        powwarm = T([1, 1], "powwarm")

        e_st = [None, T([P, NH, J, NB], "e1", bf16), T([P, NH, J, NB], "e2", bf16)]
        c_st = [None, T([P, NH, J, NB], "c1", fp16), T([P, NH, J, NB], "c2", fp16)]
        u5_t = [None, T([P, NH, 5, NB], "u5_1", bf16), T([P, NH, 5, NB], "u5_2", bf16)]
        v2_t = [None, T([P, NH, 2, NB], "v2_1", bf16), T([P, NH, 2, NB], "v2_2", bf16)]
        w1_t = [None, T([P, NH, NB], "w1_1", bf16), T([P, NH, NB], "w1_2", bf16)]
        Z_t = [None, T([P, NH, NB], "Z_1"), T([P, NH, NB], "Z_2")]
        Zr_t = [None, T([P, NH, NB], "Zr_1"), T([P, NH, NB], "Zr_2")]
        Zrb_t = [None, T([P, NH, NB], "Zrb_1", bf16), T([P, NH, NB], "Zrb_2", bf16)]

        # per-m tail tensors
        yTm_t = [T([KT, BJ], f"yTm_{m}") for m in range(3)]
        s2T_t = [T([KD, J + 1, NB], f"s2T_{m}") for m in range(3)]
        nsq_t = [T([KD, NB], f"nsq_{m}") for m in range(3)]
        uin_t = [T([KD, NB], f"uin_{m}") for m in range(3)]
        wp1_t = [T([KD, NB], f"wp1_{m}") for m in range(3)]
        r1_t = [T([KD, NB], f"r1_{m}") for m in range(3)]
        sc_t = [T([KD, NB], f"sc_{m}") for m in range(3)]
        o8T_t = [T([KD, J, NB], f"o8T_{m}") for m in range(3)]
        osqT = T([KD, J, NB], "osqT")

        # ----- inputs: xf pieces first on sync HWDGE (feeds colsums);
        # consts + xt split over scalar HWDGE and gpsimd SWDGE ----------
        for k in range(4):
            nc.sync.dma_start(
                x_f16[:, k * (NH // 4):(k + 1) * (NH // 4), :], xf_d[k]
            )
        nc.scalar.dma_start(cpack[:, :], cpack_d[:, :])
        nc.scalar.dma_start(W10flat[0:D, :], w_d[:, :])
        nc.scalar.dma_start(
            W10flat[D:DA, :],
            bass.AP(tensor=bias_d.tensor, offset=0,
                    ap=[[J * KD, 1], [1, J * KD]]),
        )
        for k in (0, 1):
            nc.scalar.dma_start(
                xT[:, k * (NH // 4):(k + 1) * (NH // 4), :], xt_d[k]
            )
        for k in (2, 3):
            nc.gpsimd.dma_start(
                xT[:, k * (NH // 4):(k + 1) * (NH // 4), :], xt_d[k]
            )

        # tiny memsets on gpsimd; pad memset on DVE (runs during first DMAs)
        nc.gpsimd.memset(phalf16[:, :], 0.5)
        nc.gpsimd.memset(halfrow[:, :], 0.5)
        nc.gpsimd.memset(tenth80[:, :], 1.0 / J)
        for m in range(3):
            nc.gpsimd.memset(s2T_t[m][:, J, :], 1e-12)
        # warm the gpsimd pow library off the critical chain
        nc.gpsimd.tensor_tensor(
            powwarm[:, :], halfrow[0:1, 0:1], halfrow[0:1, 0:1], ALU.pow
        )

        # ----- m0 colsums (DVE): per-piece partials over h, then merge ----
        for k in range(4):
            nc.vector.reduce_sum(
                part0[:, k, :],
                x_f16[:, k * (NH // 4):(k + 1) * (NH // 4), :]
                .transpose([0, 2, 1]),
                axis=mybir.AxisListType.X,
            )

        with tc.tile_pool(name="wpp", bufs=1, space="PSUM") as wpp:
            # ---- W prep: WBIGall + WT10 (early, overlaps x DMA) ----
            wb_ps = wpp.tile([KT, J * KD], f32, tag="wb", name="wb_ps")
            nc.tensor.matmul(
                wb_ps[:, :], cREP, W10flat[:, :], start=True, stop=True
            )
            nc.scalar.copy(WBIGall[:, :], wb_ps[:, :])
            wt_ps = wpp.tile([KD, J, DA], f32, tag="wt", name="wt_ps")
            for j in range(J):
                nc.tensor.transpose(
                    wt_ps[:, j, :], W10flat[:, j * KD:(j + 1) * KD], eye9
                )
            nc.scalar.copy(WT10[:, :, :], wt_ps[:, :, :])



        with tc.tile_pool(name="ypp", bufs=1, space="PSUM") as ypp:

            def tail(m, yT72, fill=()):
                """mask -> s-MMs -> squash (k-layout) -> v-MMs -> blkv.

                fill: callbacks emitting PE work injected after the s-MM /
                v-MM stages so PE stays busy during the DVE scalar chain.
                """
                yTm, s2T = yTm_t[m], s2T_t[m]
                nsqT, u, wp1, r1, scT = (
                    nsq_t[m], uin_t[m], wp1_t[m], r1_t[m], sc_t[m]
                )
                o8T = o8T_t[m]
                # mask: yTm[(b,d),(j,b')] = yT * (b==b')
                nc.vector.tensor_tensor(yTm[:, :], yT72, cBLK, ALU.mult)
                s8_ps = ypp.tile([KD, J, NB], f32, tag="ysm", name=f"s8_{m}")
                for j in range(J):
                    nc.tensor.matmul(
                        s8_ps[:, j, :],
                        WBIGall[:, j * KD:(j + 1) * KD],
                        yTm[:, j * NB:(j + 1) * NB],
                        start=True, stop=True,
                    )

                # squash scalars on [16, 8] (k-partition layout);
                # eps lives in s2T's 11th j-column (prologue memset)
                nc.scalar.activation(s2T[:, 0:J, :], s8_ps[:, :, :], AF.Square)
                nc.vector.reduce_sum(
                    nsqT[:, :],
                    s2T[:, :, :].transpose([0, 2, 1]),
                    axis=mybir.AxisListType.X,
                )
                # sc = sqrt(nsq)/(1+nsq): pow on Pool overlaps the DVE recip
                nc.gpsimd.tensor_tensor(
                    u[:, :], nsqT[:, :], phalf16[:, :], ALU.pow
                )
                nc.vector.tensor_scalar_add(wp1[:, :], nsqT[:, :], 1.0)
                nc.vector.reciprocal_approx_fast(r1[:, :], wp1[:, :])
                if len(fill) > 0:
                    fill[0]()
                nc.vector.tensor_tensor(scT[:, :], u[:, :], r1[:, :], ALU.mult)
                # o8T = s8 * sc, straight from PSUM (sc broadcast over j)
                nc.vector.tensor_tensor(
                    o8T[:, :, :],
                    s8_ps[:, :, :],
                    scT[:, :].unsqueeze(1).broadcast_to((KD, J, NB)),
                    ALU.mult,
                )
                if m == 2:
                    # final lengths: ||o||_k per (j,b) via ones-matmul
                    nc.scalar.activation(
                        osqT[:, :, :], o8T[:, :, :], AF.Square
                    )
                    ls_ps = ypp.tile([1, BJ], f32, tag="ysm", name="ls_ps")
                    nc.tensor.matmul(
                        ls_ps[:, :], onesF[0:KD, :],
                        osqT[:, :, :].rearrange("p j b -> p (j b)"),
                        start=True, stop=True,
                    )
                    nc.vector.tensor_copy(lsum[:, :], ls_ps[:, :])
                    nc.sync.dma_start(out_d[:, :], lsum[:, :])
                    return
                # vhat: vT[d,(j,b)] via per-j matmuls; accumulate; expand+mask
                vT_ps = ypp.tile([DA, BJ], f32, tag="ysm", name=f"vT_{m}")
                for j in range(J):
                    nc.tensor.matmul(
                        vT_ps[:, j * NB:(j + 1) * NB],
                        WT10[:, j, :],
                        o8T[:, j, :],
                        start=True, stop=True,
                    )
                if len(fill) > 1:
                    fill[1]()
                if m == 0:
                    nc.vector.tensor_copy(vaccT[:, :], vT_ps[:, :])
                else:
                    nc.vector.tensor_tensor(
                        vaccT[:, :], vaccT[:, :], vT_ps[:, :], ALU.add
                    )
                vd_ps = ypp.tile([KT, BJ], f32, tag="ysm", name=f"vd_{m}")
                nc.tensor.matmul(
                    vd_ps[:, :], cREP, vaccT[:, :], start=True, stop=True
                )
                nc.vector.tensor_tensor(
                    blkv_t[m][:, :], vd_ps[:, :], cBLK, ALU.mult
                )

            # ================= m = 0 (uniform c shortcut, f32) =================
            # accumulate the 4 piece-partials in PSUM via 4 tiny matmuls
            y0r_ps = ypp.tile([1, KT], f32, tag="ysm", name="y0r")
            for k in range(4):
                nc.tensor.matmul(
                    y0r_ps[:, :], onesF, part0[:, k, :],
                    start=(k == 0), stop=(k == 3),
                )
            nc.vector.tensor_copy(y0row[:, :], y0r_ps[:, :])
            yT0_ps = ypp.tile([KT, BJ], f32, tag="ysm", name="yT0")
            nc.tensor.matmul(
                yT0_ps[:, :], y0row[:, :], tenth80[:, :], start=True, stop=True
            )

            tail(0, yT0_ps[:, :])

            # ================= m = 1, 2 =================
            with tc.tile_pool(name="bwp", bufs=3, space="PSUM") as bwp:
                for m in (1, 2):
                    blkv = blkv_t[m - 1]
                    e = e_st[m]
                    cst = c_st[m]
                    u5, v2, w1 = u5_t[m], v2_t[m], w1_t[m]
                    Z, Zr, Zrb = Z_t[m], Zr_t[m], Zrb_t[m]

                    yT_ps = ypp.tile([KT, BJ], f32, tag="ysm", name=f"yT_{m}")

                    def emit_logits_exp(g):
                        # 2 waves of 8 chunks; exp per wave
                        for w in range(2):
                            hw0 = g * 16 + w * 8
                            bw = bwp.tile(
                                [P, 2, 512], f32, tag="bw",
                                name=f"bw_{m}_{hw0}",
                            )
                            for c8 in range(8):
                                h = hw0 + c8
                                off = (c8 % 4) * BJ
                                nc.tensor.matmul(
                                    bw[:, c8 // 4, off:off + BJ],
                                    xT[:, h, :], blkv[:, :],
                                    start=True, stop=True,
                                )
                            nc.scalar.activation(
                                e[:, hw0:hw0 + 8, :, :]
                                .rearrange("p (a c) j b -> p a c (j b)", a=2),
                                bw[:, :, 0:4 * BJ]
                                .rearrange("p a (c x) -> p a c x", x=BJ),
                                AF.Exp,
                            )

                    def emit_tree_c_y(g):
                        hs = slice(g * 16, g * 16 + 16)
                        QH = 16
                        nc.vector.tensor_tensor(
                            u5[:, hs, :, :], e[:, hs, 0:5, :], e[:, hs, 5:10, :],
                            ALU.add,
                        )
                        nc.vector.tensor_tensor(
                            v2[:, hs, :, :], u5[:, hs, 0:2, :], u5[:, hs, 2:4, :],
                            ALU.add,
                        )
                        nc.vector.tensor_tensor(
                            w1[:, hs, :], v2[:, hs, 0, :], v2[:, hs, 1, :],
                            ALU.add,
                        )
                        nc.vector.tensor_tensor(
                            Z[:, hs, :], w1[:, hs, :], u5[:, hs, 4, :], ALU.add
                        )
                        nc.vector.reciprocal_approx_fast(
                            Zr[:, hs, :].rearrange("p h b -> p (h b)"),
                            Z[:, hs, :].rearrange("p h b -> p (h b)"),
                        )
                        nc.gpsimd.tensor_copy(Zrb[:, hs, :], Zr[:, hs, :])
                        # c = e * Zr -> fp16; j 0:7 on DVE, 7:10 on Pool
                        nc.vector.tensor_tensor(
                            cst[:, hs, 0:7, :], e[:, hs, 0:7, :],
                            Zrb[:, hs, :].unsqueeze(2)
                            .broadcast_to((P, QH, 7, NB)),
                            ALU.mult,
                        )
                        nc.gpsimd.tensor_mul(
                            cst[:, hs, 7:10, :], e[:, hs, 7:10, :],
                            Zrb[:, hs, :].unsqueeze(2)
                            .broadcast_to((P, QH, 3, NB)),
                        )
                        for h in range(g * 16, g * 16 + 16):
                            nc.tensor.matmul(
                                yT_ps[:, :],
                                x_f16[:, h, :],
                                cst[:, h, :, :],
                                start=(h == 0), stop=(h == NH - 1),
                            )

                    emit_logits_exp(0)
                    for g in range(4):
                        if g < 3:
                            emit_logits_exp(g + 1)
                        emit_tree_c_y(g)

                    tail(m, yT_ps[:, :])

    nc.compile()
    return nc


_NC_CACHE = None


def _get_nc():
    global _NC_CACHE
    if _NC_CACHE is None:
        _NC_CACHE = _build_nc()
    return _NC_CACHE


def kernel(x, W, bias):
    x = np.asarray(x, dtype=np.float32)
    W = np.ascontiguousarray(np.asarray(W, dtype=np.float32))
    bias = np.ascontiguousarray(np.asarray(bias, dtype=np.float32))
    B = x.shape[0]
    per = B // N_CORES

    nc = _get_nc()
    in_maps = _make_in_maps(x, W, bias)
    res = bass_utils.run_bass_kernel_spmd(
        nc, in_maps, core_ids=list(range(N_CORES))
    )
    # out row is (j, b) squared lengths: [1, 80] -> [b, j], sqrt here
    outs = [np.sqrt(r["out"]).reshape(J, NB).T for r in res.results]
    return np.concatenate(outs, axis=0)


def _make_in_maps(x, W, bias):
    B = x.shape[0]
    per = B // N_CORES
    in_maps = []
    for i in range(N_CORES):
        xc = x[i * per:(i + 1) * per]                       # [8, 8192, 8]
        xa = np.concatenate(
            [xc, np.ones((per, IN, 1), np.float32)], axis=2
        ).astype(np.float16)                                # [8, 8192, 9]
        r = xa.reshape(per, P, NH, DA)
        # xf[k, p, hh, (b,d)] = xa[b, p*NH + k*8 + hh, d]
        xf = np.ascontiguousarray(
            r.transpose(1, 2, 0, 3).reshape(P, 4, NH // 4, KT)
            .transpose(1, 0, 2, 3)
        )
        # xt[k, (b,d), h, p] = xa[b, p*NH + (k*16+h), d]
        xt_full = r.transpose(0, 3, 2, 1).reshape(KT, NH, P)
        xt = np.ascontiguousarray(xt_full.reshape(KT, 4, NH // 4, P)
                                  .transpose(1, 0, 2, 3))
        in_maps.append({"xf": xf, "xt": xt, "W": W, "bias": bias})
    return in_maps


if __name__ == "__main__":
    rng = np.random.default_rng(0)
    x = rng.standard_normal((64, IN, D), dtype=np.float32)
    W = (rng.standard_normal((D, J * KD)) / np.sqrt(D)).astype(np.float32)
    bias = (rng.standard_normal(J * KD) * 0.01).astype(np.float32)
    out = kernel(x=x, W=W, bias=bias)
    print(out.shape, out[0])


# revision 48
# speedup vs baseline: 1.2980x; 1.0176x over previous
"""DenseCapsule routing kernel for Trainium2 (Bass/Tile), 8-core data-parallel.

Problem: x [64, 8192, 8], W [8, 160], bias [160] ->
  x_hat = (x @ W + bias).reshape(64, 8192, 10, 16)
  3 dynamic-routing iterations (softmax over out_num=10, weighted sum over
  in_num=8192, squash over the 10-axis, agreement update), return
  ||outputs||_2 over out_dim -> [64, 10].

Key algebra (x_hat never materialized):
  yT[(b,d), (j,b')] = sum_i x_aug[i,(b,d)] c[i,(j,b')]   (PE, masked by cBLK)
  s8T[k, (j,b)]     = per-j matmuls W_aug vs masked yT    (PE, f32)
  squash runs on the k-partition layout [16, (j,b)] so the vhat matmuls
  need no transposes; vT[d, (j,b)] via per-j matmuls; blkv = mask(cREP@vacc).
  b_logits = xT^T @ blkv, single fp16 blkv (no hi/lo split).
  softmax: exp on ACT (bf16), Z pair-tree on DVE 2x, c = e*Zr -> fp16.

Sharding: batch 64 -> 8 cores x 8 batches. Row space (b,d) = b*9+d (72 rows),
cols (j,b) = j*8+b (80). Output row [1, 80] = lengths at (j,b).
"""

from contextlib import ExitStack

import numpy as np

import concourse.bacc as bacc
import concourse.bass as bass
import concourse.mybir as mybir
import concourse.tile as tile
import concourse.bass_utils as bass_utils

f32 = mybir.dt.float32
bf16 = mybir.dt.bfloat16
fp16 = mybir.dt.float16
AF = mybir.ActivationFunctionType
ALU = mybir.AluOpType

P = 128          # SBUF partitions
NH = 64          # i-chunks per batch (8192 / 128)
NB = 8           # batches per core
D = 8            # input capsule dim
DA = 9           # augmented (+ ones column)
J = 10           # out_num
KD = 16          # out_dim
KT = NB * DA     # 72 rows (b, d)
BJ = NB * J      # 80 cols (j, b) = j*8+b
IN = 8192
N_CORES = 8


def _build_nc():
    nc = bacc.Bacc(
        "TRN2", target_bir_lowering=False, debug=False, num_devices=N_CORES
    )

    xf_d = nc.dram_tensor(
        "xf", [4, P, NH // 4, KT], fp16, kind="ExternalInput"
    ).ap()
    xt_d = nc.dram_tensor(
        "xt", [4, KT, NH // 4, P], fp16, kind="ExternalInput"
    ).ap()
    w_d = nc.dram_tensor("W", [D, J * KD], f32, kind="ExternalInput").ap()
    bias_d = nc.dram_tensor("bias", [J * KD], f32, kind="ExternalInput").ap()
    out_d = nc.dram_tensor("out", [1, BJ], f32, kind="ExternalOutput").ap()

    # ---- structural constants ----
    # cpack cols: 0:80 cBLK (rows 0:72), 80:152 cREP (rows 0:9),
    #             152:161 eye9 (rows 0:9), 161:162 ones column (all rows)
    C_BLK, C_REP, C_E9, C_ONE = 0, 80, 152, 161
    CPW = C_ONE + 1
    cpack_np = np.zeros((P, CPW), dtype=np.float32)
    for b in range(NB):
        for d in range(DA):
            for j in range(J):
                cpack_np[b * DA + d, C_BLK + j * NB + b] = 1.0
    for d in range(DA):
        for b in range(NB):
            cpack_np[d, C_REP + b * DA + d] = 1.0
    cpack_np[0:DA, C_E9:C_E9 + DA] = np.eye(DA, dtype=np.float32)
    cpack_np[:, C_ONE] = 1.0

    cpack_d = nc.inline_tensor(cpack_np, "cpack").ap()

    with tile.TileContext(nc) as tc, ExitStack() as ctx:
        sbp = ctx.enter_context(tc.tile_pool(name="sbp", bufs=1))

        def T(shape, name, dt=f32):
            return sbp.tile(shape, dt, name=name, tag=name)

        # ----- persistent SBUF tensors -----
        x_f16 = T([P, NH, KT], "x_f16", fp16)         # fp16 x_aug (host-cast)
        xT = T([KT, NH, P], "xT", fp16)               # x_aug^T (host-built)
        cpack = T([P, CPW], "cpack")
        cBLK = cpack[0:KT, C_BLK:C_BLK + BJ]
        cREP = cpack[0:DA, C_REP:C_REP + KT]
        eye9 = cpack[0:DA, C_E9:C_E9 + DA]
        onesF = cpack[:, C_ONE:C_ONE + 1]

        W10flat = T([DA, J * KD], "W10flat")          # W_aug rows d
        WBIGall = T([KT, J * KD], "WBIGall")          # W_aug repl. over b
        WT10 = T([KD, J, DA], "WT10")                 # W_aug^T per j
        part0 = T([P, 4, KT], "part0")                # m0 f32 colsum partials
        y0row = T([1, KT], "y0row")                   # m0 colsum row
        tenth80 = T([1, BJ], "tenth80")               # 0.1 expander row
        phalf16 = T([KD, NB], "phalf16")              # +0.5 (pow -> sqrt)
        halfrow = T([1, BJ], "halfrow")               # +0.5 (pow warm)
        vaccT = T([DA, BJ], "vaccT")                  # accumulated vhat^T
        blkv_t = [T([KT, BJ], f"blkv{m}", fp16) for m in range(2)]
        lsum = T([1, BJ], "lsum")
        powwarm = T([1, 1], "powwarm")

        e_st = [None, T([P, NH, J, NB], "e1", bf16), T([P, NH, J, NB], "e2", bf16)]
        c_st = [None, T([P, NH, J, NB], "c1", fp16), T([P, NH, J, NB], "c2", fp16)]
        u5_t = [None, T([P, NH, 5, NB], "u5_1", bf16), T([P, NH, 5, NB], "u5_2", bf16)]
        v2_t = [None, T([P, NH, 2, NB], "v2_1", bf16), T([P, NH, 2, NB], "v2_2", bf16)]
        w1_t = [None, T([P, NH, NB], "w1_1", bf16), T([P, NH, NB], "w1_2", bf16)]
        Z_t = [None, T([P, NH, NB], "Z_1"), T([P, NH, NB], "Z_2")]
        Zr_t = [None, T([P, NH, NB], "Zr_1"), T([P, NH, NB], "Zr_2")]
        Zrb_t = [None, T([P, NH, NB], "Zrb_1", bf16), T([P, NH, NB], "Zrb_2", bf16)]

        # per-m tail tensors
        yTm_t = [T([KT, BJ], f"yTm_{m}") for m in range(3)]
        s2T_t = [T([KD, J + 1, NB], f"s2T_{m}") for m in range(3)]
        nsq_t = [T([KD, NB], f"nsq_{m}") for m in range(3)]
        uin_t = [T([KD, NB], f"uin_{m}") for m in range(3)]
        wp1_t = [T([KD, NB], f"wp1_{m}") for m in range(3)]
        r1_t = [T([KD, NB], f"r1_{m}") for m in range(3)]
        sc_t = [T([KD, NB], f"sc_{m}") for m in range(3)]
        o8T_t = [T([KD, J, NB], f"o8T_{m}") for m in range(3)]
        osqT = T([KD, J, NB], "osqT")

        # ----- inputs: xf pieces first on sync HWDGE (feeds colsums);
        # consts + xt split over scalar HWDGE and gpsimd SWDGE ----------
        for k in range(4):
            nc.sync.dma_start(
                x_f16[:, k * (NH // 4):(k + 1) * (NH // 4), :], xf_d[k]
            )
        nc.scalar.dma_start(cpack[:, :], cpack_d[:, :])
        nc.scalar.dma_start(W10flat[0:D, :], w_d[:, :])
        nc.scalar.dma_start(
            W10flat[D:DA, :],
            bass.AP(tensor=bias_d.tensor, offset=0,
                    ap=[[J * KD, 1], [1, J * KD]]),
        )
        for k in (0, 1):
            nc.scalar.dma_start(
                xT[:, k * (NH // 4):(k + 1) * (NH // 4), :], xt_d[k]
            )
        for k in (2, 3):
            nc.gpsimd.dma_start(
                xT[:, k * (NH // 4):(k + 1) * (NH // 4), :], xt_d[k]
            )

        # tiny memsets on gpsimd; pad memset on DVE (runs during first DMAs)
        nc.gpsimd.memset(phalf16[:, :], 0.5)
        nc.gpsimd.memset(halfrow[:, :], 0.5)
        nc.gpsimd.memset(tenth80[:, :], 1.0 / J)
        for m in range(3):
            nc.gpsimd.memset(s2T_t[m][:, J, :], 1e-12)
        # warm the gpsimd pow library off the critical chain
        nc.gpsimd.tensor_tensor(
            powwarm[:, :], halfrow[0:1, 0:1], halfrow[0:1, 0:1], ALU.pow
        )

        # ----- m0 colsums (DVE): per-piece partials over h, then merge ----
        for k in range(4):
            nc.vector.reduce_sum(
                part0[:, k, :],
                x_f16[:, k * (NH // 4):(k + 1) * (NH // 4), :]
                .transpose([0, 2, 1]),
                axis=mybir.AxisListType.X,
            )

        with tc.tile_pool(name="wpp", bufs=1, space="PSUM") as wpp:
            # ---- W prep: WBIGall + WT10 (early, overlaps x DMA) ----
            wb_ps = wpp.tile([KT, J * KD], f32, tag="wb", name="wb_ps")
            nc.tensor.matmul(
                wb_ps[:, :], cREP, W10flat[:, :], start=True, stop=True
            )
            nc.scalar.copy(WBIGall[:, :], wb_ps[:, :])
            wt_ps = wpp.tile([KD, J, DA], f32, tag="wt", name="wt_ps")
            for j in range(J):
                nc.tensor.transpose(
                    wt_ps[:, j, :], W10flat[:, j * KD:(j + 1) * KD], eye9
                )
            nc.scalar.copy(WT10[:, :, :], wt_ps[:, :, :])



        with tc.tile_pool(name="ypp", bufs=1, space="PSUM") as ypp:

            def tail(m, yT72, fill=()):
                """mask -> s-MMs -> squash (k-layout) -> v-MMs -> blkv.

                fill: callbacks emitting PE work injected after the s-MM /
                v-MM stages so PE stays busy during the DVE scalar chain.
                """
                yTm, s2T = yTm_t[m], s2T_t[m]
                nsqT, u, wp1, r1, scT = (
                    nsq_t[m], uin_t[m], wp1_t[m], r1_t[m], sc_t[m]
                )
                o8T = o8T_t[m]
                # mask: yTm[(b,d),(j,b')] = yT * (b==b')
                nc.vector.tensor_tensor(yTm[:, :], yT72, cBLK, ALU.mult)
                s8_ps = ypp.tile([KD, J, NB], f32, tag="ysm", name=f"s8_{m}")
                for j in range(J):
                    nc.tensor.matmul(
                        s8_ps[:, j, :],
                        WBIGall[:, j * KD:(j + 1) * KD],
                        yTm[:, j * NB:(j + 1) * NB],
                        start=True, stop=True,
                    )

                # squash scalars on [16, 8] (k-partition layout);
                # eps lives in s2T's 11th j-column (prologue memset)
                nc.scalar.activation(s2T[:, 0:J, :], s8_ps[:, :, :], AF.Square)
                nc.vector.reduce_sum(
                    nsqT[:, :],
                    s2T[:, :, :].transpose([0, 2, 1]),
                    axis=mybir.AxisListType.X,
                )
                # sc = sqrt(nsq)/(1+nsq): pow on Pool overlaps the DVE recip
                nc.gpsimd.tensor_tensor(
                    u[:, :], nsqT[:, :], phalf16[:, :], ALU.pow
                )
                nc.vector.tensor_scalar_add(wp1[:, :], nsqT[:, :], 1.0)
                nc.vector.reciprocal_approx_fast(r1[:, :], wp1[:, :])
                if len(fill) > 0:
                    fill[0]()
                nc.vector.tensor_tensor(scT[:, :], u[:, :], r1[:, :], ALU.mult)
                # o8T = s8 * sc, straight from PSUM (sc broadcast over j)
                nc.vector.tensor_tensor(
                    o8T[:, :, :],
                    s8_ps[:, :, :],
                    scT[:, :].unsqueeze(1).broadcast_to((KD, J, NB)),
                    ALU.mult,
                )
                if m == 2:
                    # final lengths: ||o||_k per (j,b) via ones-matmul
                    nc.scalar.activation(
                        osqT[:, :, :], o8T[:, :, :], AF.Square
                    )
                    ls_ps = ypp.tile([1, BJ], f32, tag="ysm", name="ls_ps")
                    nc.tensor.matmul(
                        ls_ps[:, :], onesF[0:KD, :],
                        osqT[:, :, :].rearrange("p j b -> p (j b)"),
                        start=True, stop=True,
                    )
                    nc.vector.tensor_copy(lsum[:, :], ls_ps[:, :])
                    nc.sync.dma_start(out_d[:, :], lsum[:, :])
                    return
                # vhat: vT[d,(j,b)] via per-j matmuls; accumulate; expand+mask
                vT_ps = ypp.tile([DA, BJ], f32, tag="ysm", name=f"vT_{m}")
                for j in range(J):
                    nc.tensor.matmul(
                        vT_ps[:, j * NB:(j + 1) * NB],
                        WT10[:, j, :],
                        o8T[:, j, :],
                        start=True, stop=True,
                    )
                if len(fill) > 1:
                    fill[1]()
                if m == 0:
                    nc.vector.tensor_copy(vaccT[:, :], vT_ps[:, :])
                else:
                    nc.vector.tensor_tensor(
                        vaccT[:, :], vaccT[:, :], vT_ps[:, :], ALU.add
                    )
                vd_ps = ypp.tile([KT, BJ], f32, tag="ysm", name=f"vd_{m}")
                nc.tensor.matmul(
                    vd_ps[:, :], cREP, vaccT[:, :], start=True, stop=True
                )
                nc.vector.tensor_tensor(
                    blkv_t[m][:, :], vd_ps[:, :], cBLK, ALU.mult
                )

            # ================= m = 0 (uniform c shortcut, f32) =================
            # accumulate the 4 piece-partials in PSUM via 4 tiny matmuls
            y0r_ps = ypp.tile([1, KT], f32, tag="ysm", name="y0r")
            for k in range(4):
                nc.tensor.matmul(
                    y0r_ps[:, :], onesF, part0[:, k, :],
                    start=(k == 0), stop=(k == 3),
                )
            nc.vector.tensor_copy(y0row[:, :], y0r_ps[:, :])
            yT0_ps = ypp.tile([KT, BJ], f32, tag="ysm", name="yT0")
            nc.tensor.matmul(
                yT0_ps[:, :], y0row[:, :], tenth80[:, :], start=True, stop=True
            )

            tail(0, yT0_ps[:, :])

            # ================= m = 1, 2 =================
            with tc.tile_pool(name="bwp", bufs=3, space="PSUM") as bwp:
                for m in (1, 2):
                    blkv = blkv_t[m - 1]
                    e = e_st[m]
                    cst = c_st[m]
                    u5, v2, w1 = u5_t[m], v2_t[m], w1_t[m]
                    Z, Zr, Zrb = Z_t[m], Zr_t[m], Zrb_t[m]

                    yT_ps = ypp.tile([KT, BJ], f32, tag="ysm", name=f"yT_{m}")

                    def emit_logits_exp(h0, QH):
                        # waves of 8 chunks; exp per wave
                        for w in range(QH // 8):
                            hw0 = h0 + w * 8
                            bw = bwp.tile(
                                [P, 2, 512], f32, tag="bw",
                                name=f"bw_{m}_{hw0}",
                            )
                            for c8 in range(8):
                                h = hw0 + c8
                                off = (c8 % 4) * BJ
                                nc.tensor.matmul(
                                    bw[:, c8 // 4, off:off + BJ],
                                    xT[:, h, :], blkv[:, :],
                                    start=True, stop=True,
                                )
                            nc.scalar.activation(
                                e[:, hw0:hw0 + 8, :, :]
                                .rearrange("p (a c) j b -> p a c (j b)", a=2),
                                bw[:, :, 0:4 * BJ]
                                .rearrange("p a (c x) -> p a c x", x=BJ),
                                AF.Exp,
                            )

                    def emit_tree_c_y(h0, QH, last=False):
                        hs = slice(h0, h0 + QH)
                        nc.vector.tensor_tensor(
                            u5[:, hs, :, :], e[:, hs, 0:5, :], e[:, hs, 5:10, :],
                            ALU.add,
                        )
                        nc.vector.tensor_tensor(
                            v2[:, hs, :, :], u5[:, hs, 0:2, :], u5[:, hs, 2:4, :],
                            ALU.add,
                        )
                        nc.vector.tensor_tensor(
                            w1[:, hs, :], v2[:, hs, 0, :], v2[:, hs, 1, :],
                            ALU.add,
                        )
                        nc.vector.tensor_tensor(
                            Z[:, hs, :], w1[:, hs, :], u5[:, hs, 4, :], ALU.add
                        )
                        nc.vector.reciprocal_approx_fast(
                            Zr[:, hs, :].rearrange("p h b -> p (h b)"),
                            Z[:, hs, :].rearrange("p h b -> p (h b)"),
                        )
                        if last:
                            # endgame: keep the whole chain on DVE
                            nc.vector.tensor_copy(Zrb[:, hs, :], Zr[:, hs, :])
                            nc.vector.tensor_tensor(
                                cst[:, hs, :, :], e[:, hs, :, :],
                                Zrb[:, hs, :].unsqueeze(2)
                                .broadcast_to((P, QH, J, NB)),
                                ALU.mult,
                            )
                        else:
                            nc.gpsimd.tensor_copy(Zrb[:, hs, :], Zr[:, hs, :])
                            # c = e * Zr -> fp16; j 0:7 DVE, 7:10 Pool
                            nc.vector.tensor_tensor(
                                cst[:, hs, 0:7, :], e[:, hs, 0:7, :],
                                Zrb[:, hs, :].unsqueeze(2)
                                .broadcast_to((P, QH, 7, NB)),
                                ALU.mult,
                            )
                            nc.gpsimd.tensor_mul(
                                cst[:, hs, 7:10, :], e[:, hs, 7:10, :],
                                Zrb[:, hs, :].unsqueeze(2)
                                .broadcast_to((P, QH, 3, NB)),
                            )
                        for h in range(h0, h0 + QH):
                            nc.tensor.matmul(
                                yT_ps[:, :],
                                x_f16[:, h, :],
                                cst[:, h, :, :],
                                start=(h == 0), stop=(h == NH - 1),
                            )

                    ranges = [(0, 16), (16, 16), (32, 16), (48, 8), (56, 8)]
                    emit_logits_exp(*ranges[0])
                    for gi, r in enumerate(ranges):
                        if gi + 1 < len(ranges):
                            emit_logits_exp(*ranges[gi + 1])
                        emit_tree_c_y(*r, last=(gi == len(ranges) - 1))

                    tail(m, yT_ps[:, :])

    nc.compile()
    return nc


_NC_CACHE = None


def _get_nc():
    global _NC_CACHE
    if _NC_CACHE is None:
        _NC_CACHE = _build_nc()
    return _NC_CACHE


def kernel(x, W, bias):
    x = np.asarray(x, dtype=np.float32)
    W = np.ascontiguousarray(np.asarray(W, dtype=np.float32))
    bias = np.ascontiguousarray(np.asarray(bias, dtype=np.float32))
    B = x.shape[0]
    per = B // N_CORES

    nc = _get_nc()
    in_maps = _make_in_maps(x, W, bias)
    res = bass_utils.run_bass_kernel_spmd(
        nc, in_maps, core_ids=list(range(N_CORES))
    )
    # out row is (j, b) squared lengths: [1, 80] -> [b, j], sqrt here
    outs = [np.sqrt(r["out"]).reshape(J, NB).T for r in res.results]
    return np.concatenate(outs, axis=0)


def _make_in_maps(x, W, bias):
    B = x.shape[0]
    per = B // N_CORES
    in_maps = []
    for i in range(N_CORES):
        xc = x[i * per:(i + 1) * per]                       # [8, 8192, 8]
        xa = np.concatenate(
            [xc, np.ones((per, IN, 1), np.float32)], axis=2
        ).astype(np.float16)                                # [8, 8192, 9]
        r = xa.reshape(per, P, NH, DA)
        # xf[k, p, hh, (b,d)] = xa[b, p*NH + k*8 + hh, d]
        xf = np.ascontiguousarray(
            r.transpose(1, 2, 0, 3).reshape(P, 4, NH // 4, KT)
            .transpose(1, 0, 2, 3)
        )
        # xt[k, (b,d), h, p] = xa[b, p*NH + (k*16+h), d]
        xt_full = r.transpose(0, 3, 2, 1).reshape(KT, NH, P)
        xt = np.ascontiguousarray(xt_full.reshape(KT, 4, NH // 4, P)
                                  .transpose(1, 0, 2, 3))
        in_maps.append({"xf": xf, "xt": xt, "W": W, "bias": bias})
    return in_maps


if __name__ == "__main__":
    rng = np.random.default_rng(0)
    x = rng.standard_normal((64, IN, D), dtype=np.float32)
    W = (rng.standard_normal((D, J * KD)) / np.sqrt(D)).astype(np.float32)
    bias = (rng.standard_normal(J * KD) * 0.01).astype(np.float32)
    out = kernel(x=x, W=W, bias=bias)
    print(out.shape, out[0])
